# revision 1
# baseline (speedup 1.0000x reference)
"""Trainium2 Bass kernel for a dense transformer decoder block.

Problem: B=2, S=2048, H=2048, NH=16 (head_dim=128), FFN=8192, fp32.

Sharding (zero collectives): 8 cores = 2 batches x 4 query-chunks of 512
contiguous rows.  Every core redundantly computes LN1 + K/V projections for
its full batch (causality means late query chunks need all keys), then runs
attention for its own 512 queries against all 2048 keys (additive -1e4 mask
input reproduces the reference's causal mask exactly), followed by WO, LN2
and the FFN on its own rows.  The final output is disjoint across cores, so
the host just concatenates shards - no cross-core communication anywhere.

On-device layout is feature-major ([feature, seq] - the transpose of the
reference layout).  With weights pre-transposed on the host, every matmul in
the chain (QKV, scores^T, AV, WO, FFN1, FFN2) contracts over the partition
dimension with no on-device transposes; V is produced directly in
sequence-major layout by using the activations as the stationary operand.
LayerNorm/softmax statistics over the partition dim use ones-vector matmuls
on the PE.  All matmuls run as float32r (full-rate fp32, ~1e-4 relative
error); every matmul operand is written by its producer through a
float32r-bitcast AP, which the BIR verifier requires.
"""

import json

import numpy as np

import concourse.bass as bass
import concourse.bass2jax as bass2jax
import concourse.mybir as mybir
import concourse.tile as tile
from concourse.bass_utils import compile_bir_kernel as _orig_compile_bir_kernel
from concourse.bass_utils import run_bass_kernel_spmd

F32 = mybir.dt.float32
F32R = mybir.dt.float32r
AF = mybir.ActivationFunctionType
OP = mybir.AluOpType

B, S, H, NH, HD, FF = 2, 2048, 2048, 16, 128, 8192
P = 128
QR = 512            # query rows per core
HT = H // P         # 16 feature tiles
FT = FF // P        # 64 ffn tiles
EPS = 1e-5
NEG = -1e4

# ---------------------------------------------------------------------------
# Workaround for this container's walrus build: it supports only ONE sync
# wait per instruction, but Tile attaches several.  Rewrite the BIR just
# before walrus: an instruction with N>1 waits gets N-1 same-engine NoOps
# inserted before it, each carrying one wait (same-engine program order makes
# this equivalent).
# ---------------------------------------------------------------------------


def _split_multiwaits(bir_bytes):
    bir = json.loads(bir_bytes)
    ctr = 0
    for fn in bir.get("functions", []):
        for blk in fn.get("blocks", []):
            new = []
            for inst in blk.get("instructions", []):
                si = inst.get("sync_info")
                waits = (si or {}).get("on_wait") or []
                if len(waits) > 1:
                    for w in waits[:-1]:
                        ctr += 1
                        new.append({
                            "engine": inst["engine"],
                            "ins": [],
                            "outs": [],
                            "name": f"I-mwsplit{ctr}",
                            "opcode": "NoOp",
                            "sync_info": {"on_update": [], "on_wait": [w]},
                            "text_hint": "multiwait_split",
                        })
                    si["on_wait"] = [waits[-1]]
                new.append(inst)
            blk["instructions"] = new
    return json.dumps(bir).encode()


def _patched_compile_bir_kernel(bir_json, tmpdir, neff_name="file.neff", **kw):
    if isinstance(bir_json, str):
        bir_json = bir_json.encode()
    return _orig_compile_bir_kernel(_split_multiwaits(bir_json), tmpdir,
                                    neff_name=neff_name, **kw)


def _install_patch():
    bass2jax.compile_bir_kernel = _patched_compile_bir_kernel


def r(ap):
    """View an fp32 AP as float32r (full-rate PE mode)."""
    return ap.bitcast(F32R)


# ---------------------------------------------------------------------------
# Device program
# ---------------------------------------------------------------------------


def _ln_stats(nc, tc, pool, ones, ones_row, load_rhs, n, tag):
    """Partition-dim (feature-dim) layernorm stats via ones-matmuls.

    load_rhs: callable i -> AP [128, n], the i'th feature tile, whose
    producer already wrote it through a float32r AP.
    Returns (bmean, brstd) [128, n] tiles broadcast along partitions
    (broadcast = K=1 matmul with a [1,128] ones row as lhsT).
    """
    nch = n // 512
    mean = pool.tile([1, n], F32, tag=f"{tag}_mean", bufs=1)
    msq = pool.tile([1, n], F32, tag=f"{tag}_msq", bufs=1)
    m2 = pool.tile([1, n], F32, tag=f"{tag}_m2", bufs=1)
    rstd = pool.tile([1, n], F32, tag=f"{tag}_rstd", bufs=1)
    with tc.tile_pool(name=f"{tag}_sps", bufs=1, space="PSUM") as psum:
        mean_ps = [psum.tile([1, 512], F32, tag=f"{tag}_mps{c}",
                             name=f"{tag}_mps{c}") for c in range(nch)]
        sq_ps = [psum.tile([1, 512], F32, tag=f"{tag}_sps{c}",
                           name=f"{tag}_sps{c}") for c in range(nch)]
        for i in range(HT):
            xt = load_rhs(i)
            xsq = pool.tile([P, n], F32, tag=f"{tag}_sq", bufs=1)
            nc.scalar.activation(r(xsq[:]), xt, AF.Square)
            for c in range(nch):
                sl = slice(512 * c, 512 * (c + 1))
                nc.tensor.matmul(mean_ps[c][:], r(ones[:]), r(xt[:, sl]),
                                 start=(i == 0), stop=(i == HT - 1))
                nc.tensor.matmul(sq_ps[c][:], r(ones[:]), r(xsq[:, sl]),
                                 start=(i == 0), stop=(i == HT - 1))
        for c in range(nch):
            sl = slice(512 * c, 512 * (c + 1))
            nc.scalar.activation(r(mean[:, sl]), mean_ps[c][:], AF.Copy,
                                 scale=1.0 / H)
            nc.scalar.activation(msq[:, sl], sq_ps[c][:], AF.Copy,
                                 scale=1.0 / H)
    nc.vector.tensor_mul(m2[:], mean[:], mean[:])
    nc.vector.tensor_sub(m2[:], msq[:], m2[:])          # var
    nc.vector.tensor_scalar_add(m2[:], m2[:], EPS)
    nc.vector.reciprocal(m2[:], m2[:])                  # 1/(var+eps)
    nc.scalar.activation(r(rstd[:]), m2[:], AF.Sqrt)    # rsqrt(var+eps)
    bmean = pool.tile([P, n], F32, tag=f"{tag}_bmean", bufs=1)
    brstd = pool.tile([P, n], F32, tag=f"{tag}_brstd", bufs=1)
    with tc.tile_pool(name=f"{tag}_bps", bufs=1, space="PSUM") as bps:
        for c in range(nch):
            sl = slice(512 * c, 512 * (c + 1))
            mps = bps.tile([P, 512], F32, tag=f"{tag}_bmps{c}",
                           name=f"{tag}_bmps{c}")
            nc.tensor.matmul(mps[:], r(ones_row[:]), r(mean[:, sl]),
                             start=True, stop=True)
            nc.scalar.activation(bmean[:, sl], mps[:], AF.Copy)
            rps = bps.tile([P, 512], F32, tag=f"{tag}_brps{c}",
                           name=f"{tag}_brps{c}")
            nc.tensor.matmul(rps[:], r(ones_row[:]), r(rstd[:, sl]),
                             start=True, stop=True)
            nc.scalar.activation(brstd[:, sl], rps[:], AF.Copy)
    return bmean, brstd


def build_nc(debug_outputs=()):
    _install_patch()
    nc = bass.Bass("TRN2")

    xT = nc.dram_tensor("xT", (H, S), F32, kind="ExternalInput")
    xTq = nc.dram_tensor("xTq", (H, QR), F32, kind="ExternalInput")
    maskT = nc.dram_tensor("maskT", (S, QR), F32, kind="ExternalInput")
    ones_d = nc.dram_tensor("ones_d", (P, 1), F32, kind="ExternalInput")
    ones_r_d = nc.dram_tensor("ones_r_d", (1, P), F32, kind="ExternalInput")
    wq_t = nc.dram_tensor("wq_t", (HT, P, HT, P), F32, kind="ExternalInput")
    wk_t = nc.dram_tensor("wk_t", (HT, P, HT, P), F32, kind="ExternalInput")
    wvT = nc.dram_tensor("wvT", (H, H), F32, kind="ExternalInput")
    wo_t = nc.dram_tensor("wo_t", (HT, P, HT, P), F32, kind="ExternalInput")
    w1_t = nc.dram_tensor("w1_t", (FT, P, HT, P), F32, kind="ExternalInput")
    w2_t = nc.dram_tensor("w2_t", (HT, P, FT, P), F32, kind="ExternalInput")
    bq = nc.dram_tensor("bq", (H,), F32, kind="ExternalInput")
    bk = nc.dram_tensor("bk", (H,), F32, kind="ExternalInput")
    bv = nc.dram_tensor("bv", (H,), F32, kind="ExternalInput")
    bwo = nc.dram_tensor("bwo", (H,), F32, kind="ExternalInput")
    b1 = nc.dram_tensor("b1", (FF,), F32, kind="ExternalInput")
    b2 = nc.dram_tensor("b2", (H,), F32, kind="ExternalInput")
    ln1w = nc.dram_tensor("ln1w", (H,), F32, kind="ExternalInput")
    ln1b = nc.dram_tensor("ln1b", (H,), F32, kind="ExternalInput")
    ln2w = nc.dram_tensor("ln2w", (H,), F32, kind="ExternalInput")
    ln2b = nc.dram_tensor("ln2b", (H,), F32, kind="ExternalInput")
    outT = nc.dram_tensor("outT", (H, QR), F32, kind="ExternalOutput")

    dbg = {}
    for name, shape in dict(a=(H, S), k=(H, S), vT=(S, H), q=(H, QR),
                            av=(H, QR), h=(H, QR), g=(H, QR)).items():
        if name in debug_outputs:
            dbg[name] = nc.dram_tensor(f"dbg_{name}", shape, F32,
                                       kind="ExternalOutput")

    def dbg_dump(name, src3d):
        if name in dbg:
            for i in range(HT):
                nc.sync.dma_start(dbg[name][P * i:P * (i + 1), :],
                                  src3d[:, i, :])

    with tile.TileContext(nc) as tc:
        cm_const = tc.tile_pool(name="const", bufs=1)
        const = cm_const.__enter__()
        ones = const.tile([P, 1], F32, tag="ones")
        nc.sync.dma_start(r(ones[:]), r(ones_d[:]))
        ones_row = const.tile([1, P], F32, tag="ones_row")
        nc.sync.dma_start(r(ones_row[:]), r(ones_r_d[:]))

        def bias_tile(name, dram_t, ntiles):
            t = const.tile([P, ntiles], F32, tag=f"b_{name}")
            nc.sync.dma_start(t[:], dram_t.rearrange("(t p) -> p t", p=P))
            return t

        bq_t = bias_tile("bq", bq, HT)
        bk_t = bias_tile("bk", bk, HT)
        bv_t = bias_tile("bv", bv, HT)
        bwo_t = bias_tile("bwo", bwo, HT)
        b1_t = bias_tile("b1", b1, FT)
        b2_t = bias_tile("b2", b2, HT)
        ln1w_t = bias_tile("ln1w", ln1w, HT)
        ln1b_t = bias_tile("ln1b", ln1b, HT)
        ln2w_t = bias_tile("ln2w", ln2w, HT)
        ln2b_t = bias_tile("ln2b", ln2b, HT)

        cm_dram = tc.tile_pool(name="dram", bufs=1, space="DRAM")
        dram = cm_dram.__enter__()
        a_d = dram.tile([H, S], F32, tag="a")
        k_d = dram.tile([H, S], F32, tag="k")
        vT_d = dram.tile([S, H], F32, tag="vT")
        h_d = dram.tile([H, QR], F32, tag="h")

        # ============ S1+S2: LN1 over the full batch, fused in place =======
        cm_ares = tc.tile_pool(name="ares", bufs=1)
        arp = cm_ares.__enter__()
        a_res = arp.tile([P, HT, S], F32, tag="a_res")
        with tc.tile_pool(name="ln1", bufs=2) as lp:
            for i in range(HT):
                nc.sync.dma_start(r(a_res[:, i, :]), r(xT[P * i:P * (i + 1), :]))
            bmean, brstd = _ln_stats(nc, tc, lp, ones, ones_row,
                                     lambda i: a_res[:, i, :], S, "ln1")
            for i in range(HT):
                t1 = lp.tile([P, S], F32, tag="t1", bufs=1)
                nc.vector.tensor_sub(t1[:], a_res[:, i, :], bmean[:])
                nc.vector.tensor_mul(t1[:], t1[:], brstd[:])
                nc.vector.tensor_scalar(r(a_res[:, i, :]), t1[:],
                                        ln1w_t[:, i:i + 1], ln1b_t[:, i:i + 1],
                                        op0=OP.mult, op1=OP.add)
                nc.sync.dma_start(a_d[P * i:P * (i + 1), :], a_res[:, i, :])
                if "a" in dbg:
                    nc.sync.dma_start(dbg["a"][P * i:P * (i + 1), :],
                                      a_res[:, i, :])

        # ============ S4: K projection (a resident, WkT streamed) ===========
        with tc.tile_pool(name="kproj", bufs=2) as kp, \
             tc.tile_pool(name="kps", bufs=1, space="PSUM") as kps:
            for dM in range(HT):
                kw = kp.tile([P, HT, P], F32, tag="kw")
                nc.sync.dma_start(r(kw[:]), r(wk_t[dM]))
                ps = [kps.tile([P, 512], F32, tag=f"kp{n}", name=f"kp{n}")
                      for n in range(4)]
                for ht in range(HT):
                    for n in range(4):
                        nc.tensor.matmul(
                            ps[n][:], r(kw[:, ht, :]),
                            r(a_res[:, ht, 512 * n:512 * (n + 1)]),
                            start=(ht == 0), stop=(ht == HT - 1))
                kst = kp.tile([P, S], F32, tag="kst")
                for n in range(4):
                    nc.scalar.activation(kst[:, 512 * n:512 * (n + 1)],
                                         ps[n][:], AF.Identity,
                                         bias=bk_t[:, dM:dM + 1])
                nc.sync.dma_start(k_d[P * dM:P * (dM + 1), :], kst[:])
                if "k" in dbg:
                    nc.sync.dma_start(dbg["k"][P * dM:P * (dM + 1), :], kst[:])
        cm_ares.__exit__(None, None, None)

        # ============ S3: V^T projection (WvT resident, a streamed) =========
        with tc.tile_pool(name="vproj", bufs=2) as vp, \
             tc.tile_pool(name="wvres", bufs=1) as wvp, \
             tc.tile_pool(name="vps", bufs=1, space="PSUM") as vps:
            wv_res = wvp.tile([P, HT, H], F32, tag="wv_res")
            nc.sync.dma_start(r(wv_res[:]),
                              r(wvT.rearrange("(t p) d -> p t d", p=P)))
            for sM in range(HT):
                alh = vp.tile([P, HT, P], F32, tag="alh")
                nc.sync.dma_start(
                    r(alh[:]), r(a_d.rearrange("(t p) s -> p t s", p=P)
                                 [:, :, P * sM:P * (sM + 1)]))
                ps = [vps.tile([P, 512], F32, tag=f"vp{n}", name=f"vp{n}")
                      for n in range(4)]
                for ht in range(HT):
                    for n in range(4):
                        nc.tensor.matmul(
                            ps[n][:], r(alh[:, ht, :]),
                            r(wv_res[:, ht, 512 * n:512 * (n + 1)]),
                            start=(ht == 0), stop=(ht == HT - 1))
                vst = vp.tile([P, H], F32, tag="vst")
                for n in range(4):
                    nc.scalar.activation(vst[:, 512 * n:512 * (n + 1)],
                                         ps[n][:], AF.Copy)
                nc.sync.dma_start(vT_d[P * sM:P * (sM + 1), :], vst[:])
                if "vT" in dbg:
                    nc.sync.dma_start(dbg["vT"][P * sM:P * (sM + 1), :],
                                      vst[:])

        # ============ S4.5 + S5: LN1 on the q rows, then Q projection =======
        cm_qres = tc.tile_pool(name="qres", bufs=1)
        qres_p = cm_qres.__enter__()
        q_res = qres_p.tile([P, HT, QR], F32, tag="q_res")
        with tc.tile_pool(name="lnq", bufs=2) as lqp:
            xTq_sc = lqp.tile([P, HT, QR], F32, tag="xTq_sc", bufs=1)
            nc.sync.dma_start(r(xTq_sc[:]),
                              r(xTq.rearrange("(t p) s -> p t s", p=P)))
            bmean_q, brstd_q = _ln_stats(
                nc, tc, lqp, ones, ones_row,
                lambda i: xTq_sc[:, i, :], QR, "lnq")
            aq = lqp.tile([P, HT, QR], F32, tag="aq", bufs=1)
            for i in range(HT):
                t1 = lqp.tile([P, QR], F32, tag="t1")
                nc.vector.tensor_sub(t1[:], xTq_sc[:, i, :], bmean_q[:])
                nc.vector.tensor_mul(t1[:], t1[:], brstd_q[:])
                nc.vector.tensor_scalar(r(aq[:, i, :]), t1[:],
                                        ln1w_t[:, i:i + 1], ln1b_t[:, i:i + 1],
                                        op0=OP.mult, op1=OP.add)
            with tc.tile_pool(name="qproj", bufs=2) as qp, \
                 tc.tile_pool(name="qps", bufs=2, space="PSUM") as qps:
                for dM in range(HT):
                    qw = qp.tile([P, HT, P], F32, tag="qw")
                    nc.sync.dma_start(r(qw[:]), r(wq_t[dM]))
                    ps = qps.tile([P, QR], F32, tag="qpsum")
                    for ht in range(HT):
                        nc.tensor.matmul(ps[:], r(qw[:, ht, :]),
                                         r(aq[:, ht, :]),
                                         start=(ht == 0), stop=(ht == HT - 1))
                    nc.scalar.activation(r(q_res[:, dM, :]), ps[:],
                                         AF.Identity,
                                         bias=bq_t[:, dM:dM + 1])
                dbg_dump("q", q_res)

        # ============ S6: attention ========================================
        cm_av = tc.tile_pool(name="avres", bufs=1)
        av_p = cm_av.__enter__()
        av_res = av_p.tile([P, HT, QR], F32, tag="av_res")
        with tc.tile_pool(name="attn", bufs=2) as ap_, \
             tc.tile_pool(name="attn1", bufs=1) as ap1, \
             tc.tile_pool(name="attnps", bufs=2, space="PSUM") as aps:
            mask_res = ap1.tile([P, HT, QR], F32, tag="mask_res")
            nc.sync.dma_start(mask_res[:],
                              maskT.rearrange("(t p) s -> p t s", p=P))
            for hd_i in range(NH):
                kh = ap_.tile([P, S], F32, tag="kh")
                nc.sync.dma_start(r(kh[:]), r(k_d[P * hd_i:P * (hd_i + 1), :]))
                vh = ap_.tile([P, HT, P], F32, tag="vh")
                nc.sync.dma_start(
                    r(vh[:]), r(vT_d.rearrange("(t p) d -> p t d", p=P)
                                [:, :, P * hd_i:P * (hd_i + 1)]))
                pt = ap1.tile([P, HT, QR], F32, tag="pt")
                for kb in range(HT):
                    sp = aps.tile([P, QR], F32, tag="sp")
                    nc.tensor.matmul(sp[:], r(kh[:, P * kb:P * (kb + 1)]),
                                     r(q_res[:, hd_i, :]),
                                     start=True, stop=True)
                    ptmp = ap_.tile([P, QR], F32, tag="ptmp")
                    nc.vector.tensor_add(ptmp[:], sp[:], mask_res[:, kb, :])
                    nc.scalar.activation(r(pt[:, kb, :]), ptmp[:], AF.Exp)
                dn = aps.tile([1, QR], F32, tag="dn")
                for kb in range(HT):
                    nc.tensor.matmul(dn[:], r(ones[:]), r(pt[:, kb, :]),
                                     start=(kb == 0), stop=(kb == HT - 1))
                rec = ap_.tile([1, QR], F32, tag="rec")
                with nc.allow_low_precision(reason="f32r is fp32 bits"):
                    nc.vector.reciprocal(r(rec[:]), dn[:])
                brec_ps = aps.tile([P, QR], F32, tag="brec_ps")
                nc.tensor.matmul(brec_ps[:], r(ones_row[:]), r(rec[:]),
                                 start=True, stop=True)
                brec = ap_.tile([P, QR], F32, tag="brec")
                nc.scalar.activation(brec[:], brec_ps[:], AF.Copy)
                avp = aps.tile([P, QR], F32, tag="avp")
                for kb in range(HT):
                    nc.tensor.matmul(avp[:], r(vh[:, kb, :]), r(pt[:, kb, :]),
                                     start=(kb == 0), stop=(kb == HT - 1))
                nc.vector.tensor_mul(r(av_res[:, hd_i, :]), avp[:], brec[:])
                nc.vector.tensor_scalar_add(r(av_res[:, hd_i, :]),
                                            av_res[:, hd_i, :],
                                            bv_t[:, hd_i:hd_i + 1])
            dbg_dump("av", av_res)

        # ============ S7: WO + residual ====================================
        with tc.tile_pool(name="wo", bufs=2) as wop, \
             tc.tile_pool(name="wops", bufs=2, space="PSUM") as wops:
            for dM in range(HT):
                wot = wop.tile([P, HT, P], F32, tag="wot")
                nc.sync.dma_start(r(wot[:]), r(wo_t[dM]))
                xq_t = wop.tile([P, QR], F32, tag="xq_t")
                nc.sync.dma_start(xq_t[:], xTq[P * dM:P * (dM + 1), :])
                ps = wops.tile([P, QR], F32, tag="wopsum")
                for ht in range(HT):
                    nc.tensor.matmul(ps[:], r(wot[:, ht, :]),
                                     r(av_res[:, ht, :]),
                                     start=(ht == 0), stop=(ht == HT - 1))
                hst = wop.tile([P, QR], F32, tag="hst")
                nc.vector.scalar_tensor_tensor(
                    hst[:], ps[:], bwo_t[:, dM:dM + 1],
                    xq_t[:], op0=OP.add, op1=OP.add)
                nc.sync.dma_start(h_d[P * dM:P * (dM + 1), :], hst[:])
                if "h" in dbg:
                    nc.sync.dma_start(dbg["h"][P * dM:P * (dM + 1), :],
                                      hst[:])
        cm_av.__exit__(None, None, None)
        cm_qres.__exit__(None, None, None)

        # ============ S8: LN2 ==============================================
        cm_f = tc.tile_pool(name="fres", bufs=1)
        f_p = cm_f.__enter__()
        f_res = f_p.tile([P, FT, QR], F32, tag="f_res")
        cm_g = tc.tile_pool(name="gres", bufs=1)
        g_p = cm_g.__enter__()
        g_res = g_p.tile([P, HT, QR], F32, tag="g_res")
        with tc.tile_pool(name="ln2", bufs=2) as l2p:
            def ln2_load(i):
                ht_ = l2p.tile([P, QR], F32, tag="hl")
                nc.sync.dma_start(r(ht_[:]), r(h_d[P * i:P * (i + 1), :]))
                return ht_[:]

            bmean2, brstd2 = _ln_stats(
                nc, tc, l2p, ones, ones_row, ln2_load, QR, "ln2")
            for i in range(HT):
                hl2 = ln2_load(i)
                t1 = l2p.tile([P, QR], F32, tag="t1")
                nc.vector.tensor_sub(t1[:], hl2, bmean2[:])
                nc.vector.tensor_mul(t1[:], t1[:], brstd2[:])
                nc.vector.tensor_scalar(r(g_res[:, i, :]), t1[:],
                                        ln2w_t[:, i:i + 1], ln2b_t[:, i:i + 1],
                                        op0=OP.mult, op1=OP.add)
            dbg_dump("g", g_res)

        # ============ S9: FFN1 + gelu ======================================
        with tc.tile_pool(name="ffn1", bufs=2) as f1p, \
             tc.tile_pool(name="f1ps", bufs=2, space="PSUM") as f1ps:
            for fM in range(FT):
                w1t = f1p.tile([P, HT, P], F32, tag="w1t")
                nc.sync.dma_start(r(w1t[:]), r(w1_t[fM]))
                ps = f1ps.tile([P, QR], F32, tag="f1psum")
                for ht in range(HT):
                    nc.tensor.matmul(ps[:], r(w1t[:, ht, :]),
                                     r(g_res[:, ht, :]),
                                     start=(ht == 0), stop=(ht == HT - 1))
                nc.scalar.activation(r(f_res[:, fM, :]), ps[:], AF.Gelu,
                                     bias=b1_t[:, fM:fM + 1])
        cm_g.__exit__(None, None, None)

        # ============ S10: FFN2 + bias + residual -> output ================
        with tc.tile_pool(name="ffn2", bufs=2) as f2p, \
             tc.tile_pool(name="f2ps", bufs=2, space="PSUM") as f2ps:
            for dM in range(HT):
                ps = f2ps.tile([P, QR], F32, tag="f2psum")
                for q4 in range(4):
                    w2t = f2p.tile([P, HT, P], F32, tag="w2t")
                    nc.sync.dma_start(
                        r(w2t[:]), r(w2_t[dM][:, 16 * q4:16 * (q4 + 1), :]))
                    for ft in range(HT):
                        kk = 16 * q4 + ft
                        nc.tensor.matmul(ps[:], r(w2t[:, ft, :]),
                                         r(f_res[:, kk, :]),
                                         start=(kk == 0), stop=(kk == FT - 1))
                hfin = f2p.tile([P, QR], F32, tag="hfin")
                nc.sync.dma_start(hfin[:], h_d[P * dM:P * (dM + 1), :])
                ost = f2p.tile([P, QR], F32, tag="ost")
                nc.vector.scalar_tensor_tensor(
                    ost[:], ps[:], b2_t[:, dM:dM + 1], hfin[:],
                    op0=OP.add, op1=OP.add)
                nc.sync.dma_start(outT[P * dM:P * (dM + 1), :], ost[:])
        cm_f.__exit__(None, None, None)
        cm_dram.__exit__(None, None, None)
        cm_const.__exit__(None, None, None)

    return nc


# ---------------------------------------------------------------------------
# Host side
# ---------------------------------------------------------------------------

_CACHE = {}


def _get_nc(debug_outputs=()):
    key = tuple(sorted(debug_outputs))
    if key not in _CACHE:
        _CACHE[key] = build_nc(debug_outputs)
    return _CACHE[key]


def make_in_maps(inputs):
    x = np.asarray(inputs["x"], np.float32)
    scale = np.float32(1.0 / np.sqrt(HD))
    wqkv = np.asarray(inputs["wqkv_w"], np.float32)
    wqkv_b = np.asarray(inputs["wqkv_b"], np.float32)
    def tile_kxm(wT):
        # [K, M] -> [mM, p, kt, m2] so each [128, kt*128] lhsT load is
        # contiguous per partition
        K_, M_ = wT.shape
        return np.ascontiguousarray(
            wT.reshape(K_ // P, P, M_ // P, P).transpose(2, 1, 0, 3))

    shared = {
        "ones_d": np.ones((P, 1), np.float32),
        "ones_r_d": np.ones((1, P), np.float32),
        "wq_t": tile_kxm(wqkv[:H].T * scale),
        "wk_t": tile_kxm(np.ascontiguousarray(wqkv[H:2 * H].T)),
        "wvT": np.ascontiguousarray(wqkv[2 * H:].T),
        "wo_t": tile_kxm(np.asarray(inputs["wo_w"], np.float32).T),
        "w1_t": tile_kxm(np.asarray(inputs["w1"], np.float32).T),
        "w2_t": tile_kxm(np.asarray(inputs["w2"], np.float32).T),
        "bq": np.ascontiguousarray(wqkv_b[:H] * scale),
        "bk": np.ascontiguousarray(wqkv_b[H:2 * H]),
        "bv": np.ascontiguousarray(wqkv_b[2 * H:]),
        "bwo": np.asarray(inputs["wo_b"], np.float32),
        "b1": np.asarray(inputs["b1"], np.float32),
        "b2": np.asarray(inputs["b2"], np.float32),
        "ln1w": np.asarray(inputs["ln1_w"], np.float32),
        "ln1b": np.asarray(inputs["ln1_b"], np.float32),
        "ln2w": np.asarray(inputs["ln2_w"], np.float32),
        "ln2b": np.asarray(inputs["ln2_b"], np.float32),
    }
    kidx = np.arange(S)
    in_maps = []
    for core in range(8):
        b, c = divmod(core, 4)
        q0 = QR * c
        qidx = q0 + np.arange(QR)
        m = np.where(kidx[:, None] <= qidx[None, :], np.float32(0),
                     np.float32(NEG)).astype(np.float32)
        in_maps.append(dict(
            shared,
            xT=np.ascontiguousarray(x[b].T),
            xTq=np.ascontiguousarray(x[b, q0:q0 + QR].T),
            maskT=np.ascontiguousarray(m),
        ))
    return in_maps


def run_cores(inputs, debug_outputs=(), **run_kw):
    nc = _get_nc(debug_outputs)
    in_maps = make_in_maps(inputs)
    return nc, run_bass_kernel_spmd(nc, in_maps, core_ids=list(range(8)),
                                    **run_kw)


def kernel(**inputs):
    _, res = run_cores(inputs)
    out = np.empty((B, S, H), np.float32)
    for core in range(8):
        b, c = divmod(core, 4)
        out[b, QR * c:QR * (c + 1), :] = res.results[core]["outT"].T
    return out



# revision 16
# speedup vs baseline: 1.2526x; 1.2526x over previous
"""Trainium2 Bass kernel for a dense transformer decoder block.

Problem: B=2, S=2048, H=2048, NH=16 (head_dim=128), FFN=8192, fp32.

Sharding (zero collectives): 8 cores = 2 batches x 4 query-chunks of 512
contiguous rows.  Every core redundantly computes LN1 + K/V projections for
its full batch, then attention / WO / LN2 / FFN for its own 512 rows.  The
output is disjoint across cores; the host concatenates shards.

v2 layout: feature-major on device.  LN1 is chunked over the sequence so its
vector work overlaps K-projection matmuls; `a` (LN1 output) stays resident in
SBUF for both K and V projections; the causal mask is added on the PE via an
identity-lhsT accumulation matmul so softmax is a pure mm->exp chain;
attention is software-pipelined across heads; `h` (post-WO residual) stays
resident in SBUF; the FFN runs in bf16 (same PE rate, half the DMA).
"""

import json

import numpy as np

import concourse.bass as bass
import concourse.bass2jax as bass2jax
import concourse.mybir as mybir
import concourse.tile as tile
from concourse.bass_utils import compile_bir_kernel as _orig_compile_bir_kernel
from concourse.bass_utils import run_bass_kernel_spmd

F32 = mybir.dt.float32
F32R = mybir.dt.float32r
BF16 = mybir.dt.bfloat16
AF = mybir.ActivationFunctionType
OP = mybir.AluOpType

B, S, H, NH, HD, FF = 2, 2048, 2048, 16, 128, 8192
P = 128
QR = 512            # query rows per core
HT = H // P         # 16 feature tiles
FT = FF // P        # 64 ffn tiles
EPS = 1e-5
NEG = -1e4

# ---------------------------------------------------------------------------
# Workaround for this container's walrus build: it supports only ONE sync
# wait per instruction.  Rewrite the BIR just before walrus: an instruction
# with N>1 waits gets N-1 same-engine NoOps inserted before it, each carrying
# one wait.
# ---------------------------------------------------------------------------


def _split_multiwaits(bir_bytes):
    bir = json.loads(bir_bytes)
    ctr = 0
    for fn in bir.get("functions", []):
        for blk in fn.get("blocks", []):
            new = []
            for inst in blk.get("instructions", []):
                si = inst.get("sync_info")
                waits = (si or {}).get("on_wait") or []
                if len(waits) > 1:
                    for w in waits[:-1]:
                        ctr += 1
                        new.append({
                            "engine": inst["engine"],
                            "ins": [],
                            "outs": [],
                            "name": f"I-mwsplit{ctr}",
                            "opcode": "NoOp",
                            "sync_info": {"on_update": [], "on_wait": [w]},
                            "text_hint": "multiwait_split",
                        })
                    si["on_wait"] = [waits[-1]]
                new.append(inst)
            blk["instructions"] = new
    return json.dumps(bir).encode()


def _patched_compile_bir_kernel(bir_json, tmpdir, neff_name="file.neff", **kw):
    if isinstance(bir_json, str):
        bir_json = bir_json.encode()
    return _orig_compile_bir_kernel(_split_multiwaits(bir_json), tmpdir,
                                    neff_name=neff_name, **kw)


def _install_patch():
    bass2jax.compile_bir_kernel = _patched_compile_bir_kernel


def r(ap):
    """View an fp32 AP as float32r (full-rate PE mode)."""
    return ap.bitcast(F32R)


# ---------------------------------------------------------------------------
# Device program
# ---------------------------------------------------------------------------


def build_nc():
    _install_patch()
    nc = bass.Bass("TRN2")

    xT = nc.dram_tensor("xT", (H, S), F32, kind="ExternalInput")
    xTq = nc.dram_tensor("xTq", (H, QR), F32, kind="ExternalInput")
    maskT = nc.dram_tensor("maskT", (S, QR), F32, kind="ExternalInput")
    ident_d = nc.dram_tensor("ident_d", (P, P), F32, kind="ExternalInput")
    ones_d = nc.dram_tensor("ones_d", (P, 1), F32, kind="ExternalInput")
    ones_r_d = nc.dram_tensor("ones_r_d", (1, P), F32, kind="ExternalInput")
    wq_t = nc.dram_tensor("wq_t", (HT, P, HT, P), BF16, kind="ExternalInput")
    wk_t = nc.dram_tensor("wk_t", (HT, P, HT, P), BF16, kind="ExternalInput")
    wvT = nc.dram_tensor("wvT", (H, H), BF16, kind="ExternalInput")
    wo_t = nc.dram_tensor("wo_t", (HT, P, HT, P), BF16, kind="ExternalInput")
    w1_t = nc.dram_tensor("w1_t", (FT, P, HT, P), BF16, kind="ExternalInput")
    w2_t = nc.dram_tensor("w2_t", (HT, P, FT, P), BF16, kind="ExternalInput")
    bq = nc.dram_tensor("bq", (H,), F32, kind="ExternalInput")
    bk = nc.dram_tensor("bk", (H,), F32, kind="ExternalInput")
    bv = nc.dram_tensor("bv", (H,), F32, kind="ExternalInput")
    bwo = nc.dram_tensor("bwo", (H,), F32, kind="ExternalInput")
    b1 = nc.dram_tensor("b1", (FF,), F32, kind="ExternalInput")
    b2 = nc.dram_tensor("b2", (H,), F32, kind="ExternalInput")
    ln1w = nc.dram_tensor("ln1w", (H,), F32, kind="ExternalInput")
    ln1b = nc.dram_tensor("ln1b", (H,), F32, kind="ExternalInput")
    ln2w = nc.dram_tensor("ln2w", (H,), F32, kind="ExternalInput")
    ln2b = nc.dram_tensor("ln2b", (H,), F32, kind="ExternalInput")
    outT = nc.dram_tensor("outT", (H, QR), F32, kind="ExternalOutput")

    with tile.TileContext(nc) as tc:
        cm_const = tc.tile_pool(name="const", bufs=1)
        const = cm_const.__enter__()
        ones = const.tile([P, 1], F32, tag="ones")
        nc.sync.dma_start(r(ones[:]), r(ones_d[:]))
        ones_row = const.tile([1, P], F32, tag="ones_row")
        nc.sync.dma_start(r(ones_row[:]), r(ones_r_d[:]))
        ident = const.tile([P, P], F32, tag="ident")
        nc.sync.dma_start(r(ident[:]), r(ident_d[:]))

        def bias_tile(name, dram_t, ntiles):
            t = const.tile([P, ntiles], F32, tag=f"b_{name}")
            nc.sync.dma_start(t[:], dram_t.rearrange("(t p) -> p t", p=P))
            return t

        bq_t = bias_tile("bq", bq, HT)
        bk_t = bias_tile("bk", bk, HT)
        bv_t = bias_tile("bv", bv, HT)
        bwo_t = bias_tile("bwo", bwo, HT)
        b1_t = bias_tile("b1", b1, FT)
        b2_t = bias_tile("b2", b2, HT)
        ln1w_t = bias_tile("ln1w", ln1w, HT)
        ln1b_t = bias_tile("ln1b", ln1b, HT)
        ln2w_t = bias_tile("ln2w", ln2w, HT)
        ln2b_t = bias_tile("ln2b", ln2b, HT)

        cm_dram = tc.tile_pool(name="dram", bufs=1, space="DRAM")
        dram = cm_dram.__enter__()
        k_d = dram.tile([H, S], F32, tag="k")
        vT_d = dram.tile([S, H], F32, tag="vT")

        # q_res / av_res / h_res live across phases; h_res reuses the
        # q_res buffer (same tag, bufs=1) once attention has consumed q.
        cm_per = tc.tile_pool(name="persist", bufs=1)
        per_p = cm_per.__enter__()
        q_res = per_p.tile([P, HT, QR], F32, tag="qh", name="q_res")
        av_res = per_p.tile([P, HT, QR], BF16, tag="av", name="av_res")

        # ============ Phase A: LN1 (chunked) + K proj + lnq/Q proj =========
        # a_res holds LN1(x) in bf16 (weights for K/V/Q are bf16 too; psum
        # accumulation stays fp32 and the K/V/Q outputs are stored fp32).
        cm_ares = tc.tile_pool(name="ares", bufs=1)
        arp = cm_ares.__enter__()
        a_res = arp.tile([P, HT, S], BF16, tag="a_res")

        def ln_chunk(lp, sps, bps, stage, dst3d, dst_sl, src_dram, src_sl, n,
                     wt, bt):
            """LN a 512-col chunk: DMA f32 into `stage` [P,HT,n], compute
            stats, write the normalized result to dst3d[:, :, dst_sl] (bf16).
            """
            for i in range(HT):
                nc.sync.dma_start(r(stage[:, i, :]),
                                  r(src_dram[P * i:P * (i + 1), src_sl]))
            mean_ps = sps.tile([1, n], F32, tag="ln_mps", name="ln_mps")
            sq_ps = sps.tile([1, n], F32, tag="ln_sps", name="ln_sps")
            for i in range(HT):
                xsq = lp.tile([P, n], F32, tag="ln_sq", name="ln_sq")
                nc.scalar.activation(r(xsq[:]), stage[:, i, :], AF.Square)
                nc.tensor.matmul(mean_ps[:], r(ones[:]), r(stage[:, i, :]),
                                 start=(i == 0), stop=(i == HT - 1))
                nc.tensor.matmul(sq_ps[:], r(ones[:]), r(xsq[:]),
                                 start=(i == 0), stop=(i == HT - 1))
            mean = lp.tile([1, n], F32, tag="ln_mean", name="ln_mean")
            msq = lp.tile([1, n], F32, tag="ln_msq", name="ln_msq")
            rstd = lp.tile([1, n], F32, tag="ln_rstd", name="ln_rstd")
            nc.scalar.activation(r(mean[:]), mean_ps[:], AF.Copy, scale=1.0 / H)
            nc.scalar.activation(msq[:], sq_ps[:], AF.Copy, scale=1.0 / H)
            with nc.allow_low_precision(reason="f32r is fp32 bits"):
                nc.vector.tensor_mul(r(rstd[:]), mean[:], mean[:])
            nc.vector.tensor_sub(msq[:], msq[:], rstd[:])
            nc.vector.tensor_scalar_add(msq[:], msq[:], EPS)
            nc.vector.reciprocal(msq[:], msq[:])
            nc.scalar.activation(r(rstd[:]), msq[:], AF.Sqrt)
            bmean_ps = bps.tile([P, n], F32, tag="ln_bmps", name="ln_bmps")
            nc.tensor.matmul(bmean_ps[:], r(ones_row[:]), r(mean[:]),
                             start=True, stop=True)
            bmean = lp.tile([P, n], F32, tag="ln_bmean", name="ln_bmean")
            nc.scalar.activation(bmean[:], bmean_ps[:], AF.Copy)
            brstd_ps = bps.tile([P, n], F32, tag="ln_brps", name="ln_brps")
            nc.tensor.matmul(brstd_ps[:], r(ones_row[:]), r(rstd[:]),
                             start=True, stop=True)
            brstd = lp.tile([P, n], F32, tag="ln_brstd", name="ln_brstd")
            nc.scalar.activation(brstd[:], brstd_ps[:], AF.Copy)
            for i in range(HT):
                t1 = lp.tile([P, n], F32, tag="ln_t1", name="ln_t1")
                nc.vector.tensor_sub(t1[:], stage[:, i, :], bmean[:])
                nc.vector.tensor_mul(t1[:], t1[:], brstd[:])
                nc.vector.tensor_scalar(dst3d[:, i, dst_sl], t1[:],
                                        wt[:, i:i + 1], bt[:, i:i + 1],
                                        op0=OP.mult, op1=OP.add)

        cm_xq = tc.tile_pool(name="xqres", bufs=1)
        xq_p = cm_xq.__enter__()
        xTq_sc = xq_p.tile([P, HT, QR], BF16, tag="xTq_sc")
        with tc.tile_pool(name="ln1", bufs=1) as lp, \
             tc.tile_pool(name="lnstage", bufs=2) as stp, \
             tc.tile_pool(name="ln1ps", bufs=1, space="PSUM") as lnps, \
             tc.tile_pool(name="kqproj", bufs=2) as kp, \
             tc.tile_pool(name="kqps", bufs=2, space="PSUM") as kps:
            LC = 256
            for half in range(2):
                for c in range(4 * half, 4 * half + 4):
                    st = stp.tile([P, HT, LC], F32, tag="lnst", name="lnst")
                    ln_chunk(lp, lnps, lnps, st, a_res,
                             slice(LC * c, LC * (c + 1)),
                             xT, slice(LC * c, LC * (c + 1)), LC,
                             ln1w_t, ln1b_t)
                if half == 0:
                    # lnq: LN1 on the core's own q rows (the per-core q
                    # offset cannot be a static address in a SPMD program)
                    for c in range(2):
                        st = stp.tile([P, HT, LC], F32, tag="lnst",
                                      name="lnstq")
                        ln_chunk(lp, lnps, lnps, st, xTq_sc,
                                 slice(LC * c, LC * (c + 1)),
                                 xTq, slice(LC * c, LC * (c + 1)), LC,
                                 ln1w_t, ln1b_t)
                # K proj for this half (cols [1024*half, 1024*half+1024))
                base = 1024 * half
                for n in range(2):
                    sl = slice(base + 512 * n, base + 512 * (n + 1))
                    for dM in range(HT):
                        kw = kp.tile([P, HT, P], BF16, tag="kw",
                                     name=f"kw{half}{n}{dM}")
                        nc.sync.dma_start(kw[:], wk_t[dM])
                        ps = kps.tile([P, 512], F32, tag="kp", name="kp")
                        for ht in range(HT):
                            nc.tensor.matmul(ps[:], kw[:, ht, :],
                                             a_res[:, ht, sl],
                                             start=(ht == 0),
                                             stop=(ht == HT - 1))
                        kst = kp.tile([P, 512], F32, tag="kst", name="kst")
                        nc.scalar.activation(kst[:], ps[:], AF.Identity,
                                             bias=bk_t[:, dM:dM + 1])
                        nc.sync.dma_start(k_d[P * dM:P * (dM + 1), sl],
                                          kst[:])
                if half == 0:
                    # Q proj from the lnq output
                    for dM in range(HT):
                        qw = kp.tile([P, HT, P], BF16, tag="kw",
                                     name=f"qw{dM}")
                        nc.sync.dma_start(qw[:], wq_t[dM])
                        ps = kps.tile([P, QR], F32, tag="kp", name="qp")
                        for ht in range(HT):
                            nc.tensor.matmul(ps[:], qw[:, ht, :],
                                             xTq_sc[:, ht, :],
                                             start=(ht == 0),
                                             stop=(ht == HT - 1))
                        nc.scalar.activation(r(q_res[:, dM, :]), ps[:],
                                             AF.Identity,
                                             bias=bq_t[:, dM:dM + 1])
        cm_xq.__exit__(None, None, None)

        # ============ Phase A2: V proj (a_res resident, WvT streamed) ======
        with tc.tile_pool(name="vproj", bufs=2) as vp, \
             tc.tile_pool(name="vps", bufs=2, space="PSUM") as vps:
            for dc in range(4):
                wv_c = vp.tile([P, HT, 512], BF16, tag="wv_c")
                nc.sync.dma_start(
                    wv_c[:], wvT.rearrange("(t p) d -> p t d", p=P)
                    [:, :, 512 * dc:512 * (dc + 1)])
                for sb in range(HT):
                    ps = vps.tile([P, 512], F32, tag="vpsum")
                    for ht in range(HT):
                        nc.tensor.matmul(
                            ps[:], a_res[:, ht, P * sb:P * (sb + 1)],
                            wv_c[:, ht, :],
                            start=(ht == 0), stop=(ht == HT - 1))
                    vst = vp.tile([P, 512], F32, tag="vst")
                    nc.scalar.activation(vst[:], ps[:], AF.Copy)
                    nc.sync.dma_start(
                        vT_d[P * sb:P * (sb + 1), 512 * dc:512 * (dc + 1)],
                        vst[:])
        cm_ares.__exit__(None, None, None)

        # ============ Phase B: attention (pipelined across heads) ==========
        with tc.tile_pool(name="attn", bufs=2) as ap_, \
             tc.tile_pool(name="attn1", bufs=1) as ap1, \
             tc.tile_pool(name="attnps", bufs=2, space="PSUM") as aps:
            mask_res = ap1.tile([P, HT, QR], F32, tag="mask_res")
            nc.sync.dma_start(r(mask_res[:]),
                              r(maskT.rearrange("(t p) s -> p t s", p=P)))
            kh_t = [None] * NH
            vh_t = [None] * NH
            pt_t = [None] * NH

            def load_head(h):
                kh_t[h] = ap_.tile([P, S], F32, tag="kh", name=f"kh{h}")
                nc.sync.dma_start(r(kh_t[h][:]),
                                  r(k_d[P * h:P * (h + 1), :]))
                vh_t[h] = ap_.tile([P, HT, P], F32, tag="vh", name=f"vh{h}")
                nc.sync.dma_start(
                    r(vh_t[h][:]), r(vT_d.rearrange("(t p) d -> p t d", p=P)
                                     [:, :, P * h:P * (h + 1)]))

            def scores_exp(h):
                pt_t[h] = ap_.tile([P, HT, QR], F32, tag="pt", name=f"pt{h}")
                pt = pt_t[h]
                for kb in range(HT):
                    sp = aps.tile([P, QR], F32, tag="sp")
                    nc.tensor.matmul(sp[:], r(kh_t[h][:, P * kb:P * (kb + 1)]),
                                     r(q_res[:, h, :]), start=True, stop=False)
                    nc.tensor.matmul(sp[:], r(ident[:]),
                                     r(mask_res[:, kb, :]),
                                     start=False, stop=True)
                    nc.scalar.activation(r(pt[:, kb, :]), sp[:], AF.Exp)
                kh_t[h] = None

            def denom_av(h):
                pt = pt_t[h]
                dn = aps.tile([1, QR], F32, tag="dn")
                for kb in range(HT):
                    nc.tensor.matmul(dn[:], r(ones[:]), r(pt[:, kb, :]),
                                     start=(kb == 0), stop=(kb == HT - 1))
                rec = ap_.tile([1, QR], F32, tag="rec")
                with nc.allow_low_precision(reason="f32r is fp32 bits"):
                    nc.vector.reciprocal(r(rec[:]), dn[:])
                brec_ps = aps.tile([P, QR], F32, tag="brec_ps")
                nc.tensor.matmul(brec_ps[:], r(ones_row[:]), r(rec[:]),
                                 start=True, stop=True)
                brec = ap_.tile([P, QR], F32, tag="brec")
                nc.scalar.activation(brec[:], brec_ps[:], AF.Copy)
                avp = aps.tile([P, QR], F32, tag="avp")
                for kb in range(HT):
                    nc.tensor.matmul(avp[:], r(vh_t[h][:, kb, :]),
                                     r(pt[:, kb, :]),
                                     start=(kb == 0), stop=(kb == HT - 1))
                avn = ap_.tile([P, QR], F32, tag="avn", name=f"avn{h}")
                nc.vector.tensor_mul(avn[:], avp[:], brec[:])
                nc.vector.tensor_scalar_add(av_res[:, h, :], avn[:],
                                            bv_t[:, h:h + 1])
                vh_t[h] = None
                pt_t[h] = None

            load_head(0)
            for h in range(NH):
                if h + 1 < NH:
                    load_head(h + 1)
                scores_exp(h)
                if h > 0:
                    denom_av(h - 1)
            denom_av(NH - 1)

        # ============ Phase C: WO + residual (h stays in SBUF) =============
        h_res = per_p.tile([P, HT, QR], F32, tag="qh", name="h_res")
        with tc.tile_pool(name="wo", bufs=2) as wop, \
             tc.tile_pool(name="wops", bufs=2, space="PSUM") as wops:
            for dM in range(HT):
                wot = wop.tile([P, HT, P], BF16, tag="wot")
                nc.sync.dma_start(wot[:], wo_t[dM])
                xq_t = wop.tile([P, QR], F32, tag="xq_t")
                nc.sync.dma_start(xq_t[:], xTq[P * dM:P * (dM + 1), :])
                ps = wops.tile([P, QR], F32, tag="wopsum")
                for ht in range(HT):
                    nc.tensor.matmul(ps[:], wot[:, ht, :],
                                     av_res[:, ht, :],
                                     start=(ht == 0), stop=(ht == HT - 1))
                with nc.allow_low_precision(reason="f32r is fp32 bits"):
                    nc.vector.scalar_tensor_tensor(
                        r(h_res[:, dM, :]), ps[:], bwo_t[:, dM:dM + 1],
                        xq_t[:], op0=OP.add, op1=OP.add)

        # ============ Phase D: LN2 + FFN (bf16 weights/activations) ========
        cm_f = tc.tile_pool(name="fres", bufs=1)
        f_p = cm_f.__enter__()
        f_res = f_p.tile([P, FT, QR], BF16, tag="f_res")
        cm_g = tc.tile_pool(name="gres", bufs=1)
        g_p = cm_g.__enter__()
        g_res = g_p.tile([P, HT, QR], BF16, tag="g_res")
        with tc.tile_pool(name="ln2", bufs=2) as l2p, \
             tc.tile_pool(name="ln2s", bufs=1, space="PSUM") as l2sps, \
             tc.tile_pool(name="ln2b", bufs=1, space="PSUM") as l2bps:
            mean_ps = l2sps.tile([1, QR], F32, tag="ln2_mps")
            sq_ps = l2sps.tile([1, QR], F32, tag="ln2_sps")
            for i in range(HT):
                xsq = l2p.tile([P, QR], F32, tag="ln2_sq")
                nc.scalar.activation(r(xsq[:]), h_res[:, i, :], AF.Square)
                nc.tensor.matmul(mean_ps[:], r(ones[:]), r(h_res[:, i, :]),
                                 start=(i == 0), stop=(i == HT - 1))
                nc.tensor.matmul(sq_ps[:], r(ones[:]), r(xsq[:]),
                                 start=(i == 0), stop=(i == HT - 1))
            mean = l2p.tile([1, QR], F32, tag="ln2_mean")
            msq = l2p.tile([1, QR], F32, tag="ln2_msq")
            m2 = l2p.tile([1, QR], F32, tag="ln2_m2")
            rstd = l2p.tile([1, QR], F32, tag="ln2_rstd")
            nc.scalar.activation(r(mean[:]), mean_ps[:], AF.Copy, scale=1.0 / H)
            nc.scalar.activation(msq[:], sq_ps[:], AF.Copy, scale=1.0 / H)
            nc.vector.tensor_mul(m2[:], mean[:], mean[:])
            nc.vector.tensor_sub(m2[:], msq[:], m2[:])
            nc.vector.tensor_scalar_add(m2[:], m2[:], EPS)
            nc.vector.reciprocal(m2[:], m2[:])
            nc.scalar.activation(r(rstd[:]), m2[:], AF.Sqrt)
            bmean_ps = l2bps.tile([P, QR], F32, tag="ln2_bmps")
            nc.tensor.matmul(bmean_ps[:], r(ones_row[:]), r(mean[:]),
                             start=True, stop=True)
            bmean = l2p.tile([P, QR], F32, tag="ln2_bmean")
            nc.scalar.activation(bmean[:], bmean_ps[:], AF.Copy)
            brstd_ps = l2bps.tile([P, QR], F32, tag="ln2_brps")
            nc.tensor.matmul(brstd_ps[:], r(ones_row[:]), r(rstd[:]),
                             start=True, stop=True)
            brstd = l2p.tile([P, QR], F32, tag="ln2_brstd")
            nc.scalar.activation(brstd[:], brstd_ps[:], AF.Copy)
            for i in range(HT):
                t1 = l2p.tile([P, QR], F32, tag="ln2_t1")
                nc.vector.tensor_sub(t1[:], h_res[:, i, :], bmean[:])
                nc.vector.tensor_mul(t1[:], t1[:], brstd[:])
                nc.vector.tensor_scalar(g_res[:, i, :], t1[:],
                                        ln2w_t[:, i:i + 1], ln2b_t[:, i:i + 1],
                                        op0=OP.mult, op1=OP.add)

        with tc.tile_pool(name="ffn1", bufs=3) as f1p, \
             tc.tile_pool(name="f1ps", bufs=2, space="PSUM") as f1ps:
            for fM in range(FT):
                w1t = f1p.tile([P, HT, P], BF16, tag="w1t")
                nc.sync.dma_start(w1t[:], w1_t[fM])
                ps = f1ps.tile([P, QR], F32, tag="f1psum")
                for ht in range(HT):
                    nc.tensor.matmul(ps[:], w1t[:, ht, :], g_res[:, ht, :],
                                     start=(ht == 0), stop=(ht == HT - 1))
                nc.scalar.activation(f_res[:, fM, :], ps[:], AF.Gelu,
                                     bias=b1_t[:, fM:fM + 1])
        cm_g.__exit__(None, None, None)

        with tc.tile_pool(name="ffn2", bufs=3) as f2p, \
             tc.tile_pool(name="f2ps", bufs=2, space="PSUM") as f2ps:
            for dM in range(HT):
                ps = f2ps.tile([P, QR], F32, tag="f2psum")
                for q4 in range(4):
                    w2t = f2p.tile([P, HT, P], BF16, tag="w2t")
                    nc.sync.dma_start(
                        w2t[:], w2_t[dM][:, 16 * q4:16 * (q4 + 1), :])
                    for ft in range(HT):
                        kk = 16 * q4 + ft
                        nc.tensor.matmul(ps[:], w2t[:, ft, :],
                                         f_res[:, kk, :],
                                         start=(kk == 0), stop=(kk == FT - 1))
                ost = f2p.tile([P, QR], F32, tag="ost")
                nc.vector.scalar_tensor_tensor(
                    ost[:], ps[:], b2_t[:, dM:dM + 1], h_res[:, dM, :],
                    op0=OP.add, op1=OP.add)
                nc.sync.dma_start(outT[P * dM:P * (dM + 1), :], ost[:])
        cm_f.__exit__(None, None, None)
        cm_per.__exit__(None, None, None)
        cm_dram.__exit__(None, None, None)
        cm_const.__exit__(None, None, None)

    return nc


# ---------------------------------------------------------------------------
# Host side
# ---------------------------------------------------------------------------

_CACHE = {}


def _get_nc():
    if "nc" not in _CACHE:
        _CACHE["nc"] = build_nc()
    return _CACHE["nc"]


def make_in_maps(inputs):
    import ml_dtypes
    x = np.asarray(inputs["x"], np.float32)
    scale = np.float32(1.0 / np.sqrt(HD))
    wqkv = np.asarray(inputs["wqkv_w"], np.float32)
    wqkv_b = np.asarray(inputs["wqkv_b"], np.float32)

    def tile_kxm(wT, dt=np.float32):
        # [K, M] -> [mM, p, kt, m2] so each [128, kt*128] lhsT load is
        # contiguous per partition
        K_, M_ = wT.shape
        return np.ascontiguousarray(
            wT.reshape(K_ // P, P, M_ // P, P).transpose(2, 1, 0, 3)).astype(dt)

    shared = {
        "ones_d": np.ones((P, 1), np.float32),
        "ones_r_d": np.ones((1, P), np.float32),
        "ident_d": np.eye(P, dtype=np.float32),
        "wq_t": tile_kxm(wqkv[:H].T * scale, ml_dtypes.bfloat16),
        "wk_t": tile_kxm(np.ascontiguousarray(wqkv[H:2 * H].T),
                         ml_dtypes.bfloat16),
        "wvT": np.ascontiguousarray(wqkv[2 * H:].T).astype(ml_dtypes.bfloat16),
        "wo_t": tile_kxm(np.asarray(inputs["wo_w"], np.float32).T,
                         ml_dtypes.bfloat16),
        "w1_t": tile_kxm(np.asarray(inputs["w1"], np.float32).T,
                         ml_dtypes.bfloat16),
        "w2_t": tile_kxm(np.asarray(inputs["w2"], np.float32).T,
                         ml_dtypes.bfloat16),
        "bq": np.ascontiguousarray(wqkv_b[:H] * scale),
        "bk": np.ascontiguousarray(wqkv_b[H:2 * H]),
        "bv": np.ascontiguousarray(wqkv_b[2 * H:]),
        "bwo": np.asarray(inputs["wo_b"], np.float32),
        "b1": np.asarray(inputs["b1"], np.float32),
        "b2": np.asarray(inputs["b2"], np.float32),
        "ln1w": np.asarray(inputs["ln1_w"], np.float32),
        "ln1b": np.asarray(inputs["ln1_b"], np.float32),
        "ln2w": np.asarray(inputs["ln2_w"], np.float32),
        "ln2b": np.asarray(inputs["ln2_b"], np.float32),
    }
    kidx = np.arange(S)
    in_maps = []
    for core in range(8):
        b, c = divmod(core, 4)
        q0 = QR * c
        qidx = q0 + np.arange(QR)
        m = np.where(kidx[:, None] <= qidx[None, :], np.float32(0),
                     np.float32(NEG)).astype(np.float32)
        in_maps.append(dict(
            shared,
            xT=np.ascontiguousarray(x[b].T),
            xTq=np.ascontiguousarray(x[b, q0:q0 + QR].T),
            maskT=np.ascontiguousarray(m),
        ))
    return in_maps


def run_cores(inputs, **run_kw):
    nc = _get_nc()
    in_maps = make_in_maps(inputs)
    return nc, run_bass_kernel_spmd(nc, in_maps, core_ids=list(range(8)),
                                    **run_kw)


def kernel(**inputs):
    _, res = run_cores(inputs)
    out = np.empty((B, S, H), np.float32)
    for core in range(8):
        b, c = divmod(core, 4)
        out[b, QR * c:QR * (c + 1), :] = res.results[core]["outT"].T
    return out


# revision 19
# speedup vs baseline: 1.2967x; 1.0352x over previous
"""Trainium2 Bass kernel for a dense transformer decoder block.

Problem: B=2, S=2048, H=2048, NH=16 (head_dim=128), FFN=8192, fp32.

Sharding (zero collectives): 8 cores = 2 batches x 4 query-chunks of 512
contiguous rows.  Every core redundantly computes LN1 + K/V projections for
its full batch, then attention / WO / LN2 / FFN for its own 512 rows.  The
output is disjoint across cores; the host concatenates shards.

v2 layout: feature-major on device.  LN1 is chunked over the sequence so its
vector work overlaps K-projection matmuls; `a` (LN1 output) stays resident in
SBUF for both K and V projections; the causal mask is added on the PE via an
identity-lhsT accumulation matmul so softmax is a pure mm->exp chain;
attention is software-pipelined across heads; `h` (post-WO residual) stays
resident in SBUF; the FFN runs in bf16 (same PE rate, half the DMA).
"""

import json

import numpy as np

import concourse.bass as bass
import concourse.bass2jax as bass2jax
import concourse.mybir as mybir
import concourse.tile as tile
from concourse.bass_utils import compile_bir_kernel as _orig_compile_bir_kernel
from concourse.bass_utils import run_bass_kernel_spmd

F32 = mybir.dt.float32
F32R = mybir.dt.float32r
BF16 = mybir.dt.bfloat16
AF = mybir.ActivationFunctionType
OP = mybir.AluOpType

B, S, H, NH, HD, FF = 2, 2048, 2048, 16, 128, 8192
P = 128
QR = 512            # query rows per core
HT = H // P         # 16 feature tiles
FT = FF // P        # 64 ffn tiles
EPS = 1e-5
NEG = -1e4

# ---------------------------------------------------------------------------
# Workaround for this container's walrus build: it supports only ONE sync
# wait per instruction.  Rewrite the BIR just before walrus: an instruction
# with N>1 waits gets N-1 same-engine NoOps inserted before it, each carrying
# one wait.
# ---------------------------------------------------------------------------


def _split_multiwaits(bir_bytes):
    bir = json.loads(bir_bytes)
    ctr = 0
    for fn in bir.get("functions", []):
        for blk in fn.get("blocks", []):
            new = []
            for inst in blk.get("instructions", []):
                si = inst.get("sync_info")
                waits = (si or {}).get("on_wait") or []
                if len(waits) > 1:
                    for w in waits[:-1]:
                        ctr += 1
                        new.append({
                            "engine": inst["engine"],
                            "ins": [],
                            "outs": [],
                            "name": f"I-mwsplit{ctr}",
                            "opcode": "NoOp",
                            "sync_info": {"on_update": [], "on_wait": [w]},
                            "text_hint": "multiwait_split",
                        })
                    si["on_wait"] = [waits[-1]]
                new.append(inst)
            blk["instructions"] = new
    return json.dumps(bir).encode()


def _patched_compile_bir_kernel(bir_json, tmpdir, neff_name="file.neff", **kw):
    if isinstance(bir_json, str):
        bir_json = bir_json.encode()
    return _orig_compile_bir_kernel(_split_multiwaits(bir_json), tmpdir,
                                    neff_name=neff_name, **kw)


def _install_patch():
    bass2jax.compile_bir_kernel = _patched_compile_bir_kernel


def r(ap):
    """View an fp32 AP as float32r (full-rate PE mode)."""
    return ap.bitcast(F32R)


# ---------------------------------------------------------------------------
# Device program
# ---------------------------------------------------------------------------


def build_nc():
    _install_patch()
    nc = bass.Bass("TRN2")

    xT = nc.dram_tensor("xT", (H, S), F32, kind="ExternalInput")
    xTq = nc.dram_tensor("xTq", (H, QR), F32, kind="ExternalInput")
    maskT = nc.dram_tensor("maskT", (S, QR), BF16, kind="ExternalInput")
    ident_d = nc.dram_tensor("ident_d", (P, P), BF16, kind="ExternalInput")
    ones_bf_d = nc.dram_tensor("ones_bf_d", (P, 1), BF16, kind="ExternalInput")
    ones_d = nc.dram_tensor("ones_d", (P, 1), F32, kind="ExternalInput")
    ones_r_d = nc.dram_tensor("ones_r_d", (1, P), F32, kind="ExternalInput")
    wq_t = nc.dram_tensor("wq_t", (HT, P, HT, P), BF16, kind="ExternalInput")
    wk_t = nc.dram_tensor("wk_t", (HT, P, HT, P), BF16, kind="ExternalInput")
    wvT = nc.dram_tensor("wvT", (H, H), BF16, kind="ExternalInput")
    wo_t = nc.dram_tensor("wo_t", (HT, P, HT, P), BF16, kind="ExternalInput")
    w1_t = nc.dram_tensor("w1_t", (FT, P, HT, P), BF16, kind="ExternalInput")
    w2_t = nc.dram_tensor("w2_t", (HT, P, FT, P), BF16, kind="ExternalInput")
    bq = nc.dram_tensor("bq", (H,), F32, kind="ExternalInput")
    bk = nc.dram_tensor("bk", (H,), F32, kind="ExternalInput")
    bv = nc.dram_tensor("bv", (H,), F32, kind="ExternalInput")
    bwo = nc.dram_tensor("bwo", (H,), F32, kind="ExternalInput")
    b1 = nc.dram_tensor("b1", (FF,), F32, kind="ExternalInput")
    b2 = nc.dram_tensor("b2", (H,), F32, kind="ExternalInput")
    outT = nc.dram_tensor("outT", (H, QR), F32, kind="ExternalOutput")

    with tile.TileContext(nc) as tc:
        cm_const = tc.tile_pool(name="const", bufs=1)
        const = cm_const.__enter__()
        ones = const.tile([P, 1], F32, tag="ones")
        nc.sync.dma_start(r(ones[:]), r(ones_d[:]))
        ones_row = const.tile([1, P], F32, tag="ones_row")
        nc.sync.dma_start(r(ones_row[:]), r(ones_r_d[:]))
        ident = const.tile([P, P], BF16, tag="ident")
        nc.sync.dma_start(ident[:], ident_d[:])
        ones_bf = const.tile([P, 1], BF16, tag="ones_bf")
        nc.sync.dma_start(ones_bf[:], ones_bf_d[:])

        def bias_tile(name, dram_t, ntiles):
            t = const.tile([P, ntiles], F32, tag=f"b_{name}")
            nc.sync.dma_start(t[:], dram_t.rearrange("(t p) -> p t", p=P))
            return t

        bq_t = bias_tile("bq", bq, HT)
        bk_t = bias_tile("bk", bk, HT)
        bv_t = bias_tile("bv", bv, HT)
        bwo_t = bias_tile("bwo", bwo, HT)
        b1_t = bias_tile("b1", b1, FT)
        b2_t = bias_tile("b2", b2, HT)

        cm_dram = tc.tile_pool(name="dram", bufs=1, space="DRAM")
        dram = cm_dram.__enter__()
        k_d = dram.tile([H, S], BF16, tag="k")
        vT_d = dram.tile([S, H], BF16, tag="vT")

        # q_res / av_res / h_res live across phases; h_res reuses the
        # q_res buffer (same tag, bufs=1) once attention has consumed q.
        cm_per = tc.tile_pool(name="persist", bufs=1)
        per_p = cm_per.__enter__()
        q_res = per_p.tile([P, HT, QR], BF16, tag="q", name="q_res")
        av_res = per_p.tile([P, HT, QR], BF16, tag="av", name="av_res")

        # ============ Phase A: LN1 (chunked) + K proj + lnq/Q proj =========
        # a_res holds LN1(x) in bf16 (weights for K/V/Q are bf16 too; psum
        # accumulation stays fp32 and the K/V/Q outputs are stored fp32).
        cm_ares = tc.tile_pool(name="ares", bufs=1)
        arp = cm_ares.__enter__()
        a_res = arp.tile([P, HT, S], BF16, tag="a_res")

        def ln_chunk(lp, sps, bps, stage, dst3d, dst_sl, src_dram, src_sl, n):
            """LN a 512-col chunk: DMA f32 into `stage` [P,HT,n], compute
            stats, write the normalized result to dst3d[:, :, dst_sl] (bf16).
            """
            for i in range(HT):
                nc.sync.dma_start(r(stage[:, i, :]),
                                  r(src_dram[P * i:P * (i + 1), src_sl]))
            mean_ps = sps.tile([1, n], F32, tag="ln_mps", name="ln_mps")
            sq_ps = sps.tile([1, n], F32, tag="ln_sps", name="ln_sps")
            for i in range(HT):
                xsq = lp.tile([P, n], F32, tag="ln_sq", name="ln_sq")
                nc.scalar.activation(r(xsq[:]), stage[:, i, :], AF.Square)
                nc.tensor.matmul(mean_ps[:], r(ones[:]), r(stage[:, i, :]),
                                 start=(i == 0), stop=(i == HT - 1))
                nc.tensor.matmul(sq_ps[:], r(ones[:]), r(xsq[:]),
                                 start=(i == 0), stop=(i == HT - 1))
            mean = lp.tile([1, n], F32, tag="ln_mean", name="ln_mean")
            msq = lp.tile([1, n], F32, tag="ln_msq", name="ln_msq")
            rstd = lp.tile([1, n], F32, tag="ln_rstd", name="ln_rstd")
            nc.scalar.activation(r(mean[:]), mean_ps[:], AF.Copy, scale=1.0 / H)
            nc.scalar.activation(msq[:], sq_ps[:], AF.Copy, scale=1.0 / H)
            with nc.allow_low_precision(reason="f32r is fp32 bits"):
                nc.vector.tensor_mul(r(rstd[:]), mean[:], mean[:])
            nc.vector.tensor_sub(msq[:], msq[:], rstd[:])
            nc.vector.tensor_scalar_add(msq[:], msq[:], EPS)
            nc.vector.reciprocal(msq[:], msq[:])
            nc.scalar.activation(r(rstd[:]), msq[:], AF.Sqrt)
            bmean_ps = bps.tile([P, n], F32, tag="ln_bmps", name="ln_bmps")
            nc.tensor.matmul(bmean_ps[:], r(ones_row[:]), r(mean[:]),
                             start=True, stop=True)
            bmean = lp.tile([P, n], F32, tag="ln_bmean", name="ln_bmean")
            nc.scalar.activation(bmean[:], bmean_ps[:], AF.Copy)
            brstd_ps = bps.tile([P, n], F32, tag="ln_brps", name="ln_brps")
            nc.tensor.matmul(brstd_ps[:], r(ones_row[:]), r(rstd[:]),
                             start=True, stop=True)
            brstd = lp.tile([P, n], F32, tag="ln_brstd", name="ln_brstd")
            nc.scalar.activation(brstd[:], brstd_ps[:], AF.Copy)
            for i in range(HT):
                t1 = lp.tile([P, n], F32, tag="ln_t1", name="ln_t1")
                nc.vector.tensor_sub(t1[:], stage[:, i, :], bmean[:])
                nc.vector.tensor_mul(dst3d[:, i, dst_sl], t1[:], brstd[:])

        cm_xq = tc.tile_pool(name="xqres", bufs=1)
        xq_p = cm_xq.__enter__()
        xTq_sc = xq_p.tile([P, HT, QR], BF16, tag="xTq_sc")
        with tc.tile_pool(name="ln1", bufs=1) as lp, \
             tc.tile_pool(name="lnstage", bufs=2) as stp, \
             tc.tile_pool(name="ln1ps", bufs=1, space="PSUM") as lnps, \
             tc.tile_pool(name="kqproj", bufs=2) as kp, \
             tc.tile_pool(name="kqps", bufs=2, space="PSUM") as kps:
            LC = 256
            for half in range(2):
                for c in range(4 * half, 4 * half + 4):
                    st = stp.tile([P, HT, LC], F32, tag="lnst", name="lnst")
                    ln_chunk(lp, lnps, lnps, st, a_res,
                             slice(LC * c, LC * (c + 1)),
                             xT, slice(LC * c, LC * (c + 1)), LC)
                if half == 0:
                    # lnq: LN1 on the core's own q rows (the per-core q
                    # offset cannot be a static address in a SPMD program)
                    for c in range(2):
                        st = stp.tile([P, HT, LC], F32, tag="lnst",
                                      name="lnstq")
                        ln_chunk(lp, lnps, lnps, st, xTq_sc,
                                 slice(LC * c, LC * (c + 1)),
                                 xTq, slice(LC * c, LC * (c + 1)), LC)
                # K proj for this half (cols [1024*half, 1024*half+1024))
                base = 1024 * half
                for n in range(2):
                    sl = slice(base + 512 * n, base + 512 * (n + 1))
                    for dM in range(HT):
                        kw = kp.tile([P, HT, P], BF16, tag="kw",
                                     name=f"kw{half}{n}{dM}")
                        nc.sync.dma_start(kw[:], wk_t[dM])
                        ps = kps.tile([P, 512], F32, tag="kp", name="kp")
                        for ht in range(HT):
                            nc.tensor.matmul(ps[:], kw[:, ht, :],
                                             a_res[:, ht, sl],
                                             start=(ht == 0),
                                             stop=(ht == HT - 1))
                        kst = kp.tile([P, 512], BF16, tag="kst", name="kst")
                        nc.scalar.activation(kst[:], ps[:], AF.Identity,
                                             bias=bk_t[:, dM:dM + 1])
                        nc.sync.dma_start(k_d[P * dM:P * (dM + 1), sl],
                                          kst[:])
                if half == 0:
                    # Q proj from the lnq output
                    for dM in range(HT):
                        qw = kp.tile([P, HT, P], BF16, tag="kw",
                                     name=f"qw{dM}")
                        nc.sync.dma_start(qw[:], wq_t[dM])
                        ps = kps.tile([P, QR], F32, tag="kp", name="qp")
                        for ht in range(HT):
                            nc.tensor.matmul(ps[:], qw[:, ht, :],
                                             xTq_sc[:, ht, :],
                                             start=(ht == 0),
                                             stop=(ht == HT - 1))
                        nc.scalar.activation(q_res[:, dM, :], ps[:],
                                             AF.Identity,
                                             bias=bq_t[:, dM:dM + 1])
        cm_xq.__exit__(None, None, None)

        # ============ Phase A2: V proj (a_res resident, WvT streamed) ======
        with tc.tile_pool(name="vproj", bufs=2) as vp, \
             tc.tile_pool(name="vps", bufs=2, space="PSUM") as vps:
            for dc in range(4):
                wv_c = vp.tile([P, HT, 512], BF16, tag="wv_c")
                nc.sync.dma_start(
                    wv_c[:], wvT.rearrange("(t p) d -> p t d", p=P)
                    [:, :, 512 * dc:512 * (dc + 1)])
                for sb in range(HT):
                    ps = vps.tile([P, 512], F32, tag="vpsum")
                    for ht in range(HT):
                        nc.tensor.matmul(
                            ps[:], a_res[:, ht, P * sb:P * (sb + 1)],
                            wv_c[:, ht, :],
                            start=(ht == 0), stop=(ht == HT - 1))
                    vst = vp.tile([P, 512], BF16, tag="vst")
                    nc.scalar.activation(vst[:], ps[:], AF.Copy)
                    nc.sync.dma_start(
                        vT_d[P * sb:P * (sb + 1), 512 * dc:512 * (dc + 1)],
                        vst[:])
        cm_ares.__exit__(None, None, None)

        # ============ Phase B: attention (pipelined across heads) ==========
        with tc.tile_pool(name="attn", bufs=2) as ap_, \
             tc.tile_pool(name="attn1", bufs=1) as ap1, \
             tc.tile_pool(name="attnps", bufs=2, space="PSUM") as aps:
            mask_res = ap1.tile([P, HT, QR], BF16, tag="mask_res")
            nc.sync.dma_start(mask_res[:],
                              maskT.rearrange("(t p) s -> p t s", p=P))
            kh_t = [None] * NH
            vh_t = [None] * NH
            pt_t = [None] * NH

            def load_head(h):
                kh_t[h] = ap_.tile([P, S], BF16, tag="kh", name=f"kh{h}")
                nc.sync.dma_start(kh_t[h][:], k_d[P * h:P * (h + 1), :])
                vh_t[h] = ap_.tile([P, HT, P], BF16, tag="vh", name=f"vh{h}")
                nc.sync.dma_start(
                    vh_t[h][:], vT_d.rearrange("(t p) d -> p t d", p=P)
                    [:, :, P * h:P * (h + 1)])

            def scores_exp(h):
                pt_t[h] = ap_.tile([P, HT, QR], BF16, tag="pt", name=f"pt{h}")
                pt = pt_t[h]
                for kb in range(HT):
                    sp = aps.tile([P, QR], F32, tag="sp")
                    nc.tensor.matmul(sp[:], kh_t[h][:, P * kb:P * (kb + 1)],
                                     q_res[:, h, :], start=True, stop=False)
                    nc.tensor.matmul(sp[:], ident[:], mask_res[:, kb, :],
                                     start=False, stop=True)
                    nc.scalar.activation(pt[:, kb, :], sp[:], AF.Exp)
                kh_t[h] = None

            def denom_av(h):
                pt = pt_t[h]
                dn = aps.tile([1, QR], F32, tag="dn")
                for kb in range(HT):
                    nc.tensor.matmul(dn[:], ones_bf[:], pt[:, kb, :],
                                     start=(kb == 0), stop=(kb == HT - 1))
                rec = ap_.tile([1, QR], F32, tag="rec")
                with nc.allow_low_precision(reason="f32r is fp32 bits"):
                    nc.vector.reciprocal(r(rec[:]), dn[:])
                brec_ps = aps.tile([P, QR], F32, tag="brec_ps")
                nc.tensor.matmul(brec_ps[:], r(ones_row[:]), r(rec[:]),
                                 start=True, stop=True)
                brec = ap_.tile([P, QR], F32, tag="brec")
                nc.scalar.activation(brec[:], brec_ps[:], AF.Copy)
                avp = aps.tile([P, QR], F32, tag="avp")
                for kb in range(HT):
                    nc.tensor.matmul(avp[:], vh_t[h][:, kb, :],
                                     pt[:, kb, :],
                                     start=(kb == 0), stop=(kb == HT - 1))
                avn = ap_.tile([P, QR], F32, tag="avn", name=f"avn{h}")
                nc.vector.tensor_mul(avn[:], avp[:], brec[:])
                nc.vector.tensor_scalar_add(av_res[:, h, :], avn[:],
                                            bv_t[:, h:h + 1])
                vh_t[h] = None
                pt_t[h] = None

            load_head(0)
            for h in range(NH):
                if h + 1 < NH:
                    load_head(h + 1)
                scores_exp(h)
                if h > 0:
                    denom_av(h - 1)
            denom_av(NH - 1)

        # ============ Phase C: WO + residual + LN2 (stats interleaved) =====
        h_res = per_p.tile([P, HT, QR], F32, tag="h", name="h_res")
        cm_f = tc.tile_pool(name="fres", bufs=1)
        f_p = cm_f.__enter__()
        f_res = f_p.tile([P, FT, QR], BF16, tag="f_res")
        cm_g = tc.tile_pool(name="gres", bufs=1)
        g_p = cm_g.__enter__()
        g_res = g_p.tile([P, HT, QR], BF16, tag="g_res")
        with tc.tile_pool(name="wo", bufs=2) as wop, \
             tc.tile_pool(name="wops", bufs=2, space="PSUM") as wops, \
             tc.tile_pool(name="ln2", bufs=1) as l2p, \
             tc.tile_pool(name="ln2ps", bufs=1, space="PSUM") as l2ps:
            mean_ps = l2ps.tile([1, QR], F32, tag="ln2_mps")
            sq_ps = l2ps.tile([1, QR], F32, tag="ln2_sps")
            for dM in range(HT):
                wot = wop.tile([P, HT, P], BF16, tag="wot")
                nc.sync.dma_start(wot[:], wo_t[dM])
                xq_t = wop.tile([P, QR], F32, tag="xq_t")
                nc.sync.dma_start(xq_t[:], xTq[P * dM:P * (dM + 1), :])
                ps = wops.tile([P, QR], F32, tag="wopsum")
                for ht in range(HT):
                    nc.tensor.matmul(ps[:], wot[:, ht, :],
                                     av_res[:, ht, :],
                                     start=(ht == 0), stop=(ht == HT - 1))
                with nc.allow_low_precision(reason="f32r is fp32 bits"):
                    nc.vector.scalar_tensor_tensor(
                        r(h_res[:, dM, :]), ps[:], bwo_t[:, dM:dM + 1],
                        xq_t[:], op0=OP.add, op1=OP.add)
                xsq = l2p.tile([P, QR], F32, tag="ln2_sq", name="ln2_sq")
                nc.scalar.activation(r(xsq[:]), h_res[:, dM, :], AF.Square)
                nc.tensor.matmul(mean_ps[:], r(ones[:]), r(h_res[:, dM, :]),
                                 start=(dM == 0), stop=(dM == HT - 1))
                nc.tensor.matmul(sq_ps[:], r(ones[:]), r(xsq[:]),
                                 start=(dM == 0), stop=(dM == HT - 1))
            mean = l2p.tile([1, QR], F32, tag="ln2_mean")
            msq = l2p.tile([1, QR], F32, tag="ln2_msq")
            rstd = l2p.tile([1, QR], F32, tag="ln2_rstd")
            nc.scalar.activation(r(mean[:]), mean_ps[:], AF.Copy, scale=1.0 / H)
            nc.scalar.activation(msq[:], sq_ps[:], AF.Copy, scale=1.0 / H)
            with nc.allow_low_precision(reason="f32r is fp32 bits"):
                nc.vector.tensor_mul(r(rstd[:]), mean[:], mean[:])
            nc.vector.tensor_sub(msq[:], msq[:], rstd[:])
            nc.vector.tensor_scalar_add(msq[:], msq[:], EPS)
            nc.vector.reciprocal(msq[:], msq[:])
            nc.scalar.activation(r(rstd[:]), msq[:], AF.Sqrt)
            bmean_ps = wops.tile([P, QR], F32, tag="wopsum", name="l2bm")
            nc.tensor.matmul(bmean_ps[:], r(ones_row[:]), r(mean[:]),
                             start=True, stop=True)
            bmean = l2p.tile([P, QR], F32, tag="ln2_bmean")
            nc.scalar.activation(bmean[:], bmean_ps[:], AF.Copy)
            brstd_ps = wops.tile([P, QR], F32, tag="wopsum", name="l2br")
            nc.tensor.matmul(brstd_ps[:], r(ones_row[:]), r(rstd[:]),
                             start=True, stop=True)
            brstd = l2p.tile([P, QR], F32, tag="ln2_brstd")
            nc.scalar.activation(brstd[:], brstd_ps[:], AF.Copy)
            for i in range(HT):
                t1 = l2p.tile([P, QR], F32, tag="ln2_t1", name="ln2_t1")
                nc.vector.tensor_sub(t1[:], h_res[:, i, :], bmean[:])
                nc.vector.tensor_mul(g_res[:, i, :], t1[:], brstd[:])

        with tc.tile_pool(name="ffn1", bufs=3) as f1p, \
             tc.tile_pool(name="f1ps", bufs=2, space="PSUM") as f1ps:
            for fM in range(FT):
                w1t = f1p.tile([P, HT, P], BF16, tag="w1t")
                nc.sync.dma_start(w1t[:], w1_t[fM])
                ps = f1ps.tile([P, QR], F32, tag="f1psum")
                for ht in range(HT):
                    nc.tensor.matmul(ps[:], w1t[:, ht, :], g_res[:, ht, :],
                                     start=(ht == 0), stop=(ht == HT - 1))
                nc.scalar.activation(f_res[:, fM, :], ps[:], AF.Gelu,
                                     bias=b1_t[:, fM:fM + 1])
        cm_g.__exit__(None, None, None)

        with tc.tile_pool(name="ffn2", bufs=3) as f2p, \
             tc.tile_pool(name="f2ps", bufs=2, space="PSUM") as f2ps:
            for dM in range(HT):
                ps = f2ps.tile([P, QR], F32, tag="f2psum")
                for q4 in range(4):
                    w2t = f2p.tile([P, HT, P], BF16, tag="w2t")
                    nc.sync.dma_start(
                        w2t[:], w2_t[dM][:, 16 * q4:16 * (q4 + 1), :])
                    for ft in range(HT):
                        kk = 16 * q4 + ft
                        nc.tensor.matmul(ps[:], w2t[:, ft, :],
                                         f_res[:, kk, :],
                                         start=(kk == 0), stop=(kk == FT - 1))
                ost = f2p.tile([P, QR], F32, tag="ost")
                nc.vector.scalar_tensor_tensor(
                    ost[:], ps[:], b2_t[:, dM:dM + 1], h_res[:, dM, :],
                    op0=OP.add, op1=OP.add)
                nc.sync.dma_start(outT[P * dM:P * (dM + 1), :], ost[:])
        cm_f.__exit__(None, None, None)
        cm_per.__exit__(None, None, None)
        cm_dram.__exit__(None, None, None)
        cm_const.__exit__(None, None, None)

    return nc


# ---------------------------------------------------------------------------
# Host side
# ---------------------------------------------------------------------------

_CACHE = {}


def _get_nc():
    if "nc" not in _CACHE:
        _CACHE["nc"] = build_nc()
    return _CACHE["nc"]


def make_in_maps(inputs):
    import ml_dtypes
    BF = ml_dtypes.bfloat16
    x = np.asarray(inputs["x"], np.float32)
    scale = np.float32(1.0 / np.sqrt(HD))
    wqkv = np.asarray(inputs["wqkv_w"], np.float32)
    wqkv_b = np.asarray(inputs["wqkv_b"], np.float32)
    ln1w = np.asarray(inputs["ln1_w"], np.float32)
    ln1b = np.asarray(inputs["ln1_b"], np.float32)
    ln2w = np.asarray(inputs["ln2_w"], np.float32)
    ln2b = np.asarray(inputs["ln2_b"], np.float32)
    w1 = np.asarray(inputs["w1"], np.float32)
    wq, wk, wv = wqkv[:H], wqkv[H:2 * H], wqkv[2 * H:]

    def tile_kxm(wT, dt=np.float32):
        # [K, M] -> [mM, p, kt, m2] so each [128, kt*128] lhsT load is
        # contiguous per partition
        K_, M_ = wT.shape
        return np.ascontiguousarray(
            wT.reshape(K_ // P, P, M_ // P, P).transpose(2, 1, 0, 3)).astype(dt)

    # LN affine params are folded into the downstream projections:
    # W @ (y*w + b) = (W*diag(w)) @ y + W @ b   (exact algebra)
    shared = {
        "ones_d": np.ones((P, 1), np.float32),
        "ones_bf_d": np.ones((P, 1), BF),
        "ones_r_d": np.ones((1, P), np.float32),
        "ident_d": np.eye(P, dtype=np.float32).astype(BF),
        "wq_t": tile_kxm(wq.T * ln1w[:, None] * scale, BF),
        "wk_t": tile_kxm(wk.T * ln1w[:, None], BF),
        "wvT": np.ascontiguousarray(wv.T * ln1w[:, None]).astype(BF),
        "wo_t": tile_kxm(np.asarray(inputs["wo_w"], np.float32).T, BF),
        "w1_t": tile_kxm(w1.T * ln2w[:, None], BF),
        "w2_t": tile_kxm(np.asarray(inputs["w2"], np.float32).T, BF),
        "bq": np.ascontiguousarray((wqkv_b[:H] + wq @ ln1b) * scale),
        "bk": np.ascontiguousarray(wqkv_b[H:2 * H] + wk @ ln1b),
        "bv": np.ascontiguousarray(wqkv_b[2 * H:] + wv @ ln1b),
        "bwo": np.asarray(inputs["wo_b"], np.float32),
        "b1": np.asarray(inputs["b1"], np.float32) + w1 @ ln2b,
        "b2": np.asarray(inputs["b2"], np.float32),
    }
    kidx = np.arange(S)
    in_maps = []
    for core in range(8):
        b, c = divmod(core, 4)
        q0 = QR * c
        qidx = q0 + np.arange(QR)
        m = np.where(kidx[:, None] <= qidx[None, :], np.float32(0),
                     np.float32(NEG)).astype(BF)
        in_maps.append(dict(
            shared,
            xT=np.ascontiguousarray(x[b].T),
            xTq=np.ascontiguousarray(x[b, q0:q0 + QR].T),
            maskT=np.ascontiguousarray(m),
        ))
    return in_maps


def run_cores(inputs, **run_kw):
    nc = _get_nc()
    in_maps = make_in_maps(inputs)
    return nc, run_bass_kernel_spmd(nc, in_maps, core_ids=list(range(8)),
                                    **run_kw)


def kernel(**inputs):
    _, res = run_cores(inputs)
    out = np.empty((B, S, H), np.float32)
    for core in range(8):
        b, c = divmod(core, 4)
        out[b, QR * c:QR * (c + 1), :] = res.results[core]["outT"].T
    return out


# revision 20
# speedup vs baseline: 1.5100x; 1.1645x over previous
"""Trainium2 Bass kernel for a dense transformer decoder block.

Problem: B=2, S=2048, H=2048, NH=16 (head_dim=128), FFN=8192, fp32.

Sharding (zero collectives): 8 cores = 2 batches x 4 query-chunks of 512
contiguous rows.  Every core redundantly computes LN1 + K/V projections for
its full batch, then attention / WO / LN2 / FFN for its own 512 rows.  The
output is disjoint across cores; the host concatenates shards.

v2 layout: feature-major on device.  LN1 is chunked over the sequence so its
vector work overlaps K-projection matmuls; `a` (LN1 output) stays resident in
SBUF for both K and V projections; the causal mask is added on the PE via an
identity-lhsT accumulation matmul so softmax is a pure mm->exp chain;
attention is software-pipelined across heads; `h` (post-WO residual) stays
resident in SBUF; the FFN runs in bf16 (same PE rate, half the DMA).
"""

import json

import numpy as np

import concourse.bass as bass
import concourse.bass2jax as bass2jax
import concourse.mybir as mybir
import concourse.tile as tile
from concourse.bass_utils import compile_bir_kernel as _orig_compile_bir_kernel
from concourse.bass_utils import run_bass_kernel_spmd

F32 = mybir.dt.float32
F32R = mybir.dt.float32r
BF16 = mybir.dt.bfloat16
AF = mybir.ActivationFunctionType
OP = mybir.AluOpType

B, S, H, NH, HD, FF = 2, 2048, 2048, 16, 128, 8192
P = 128
QR = 512            # query rows per core
HT = H // P         # 16 feature tiles
FT = FF // P        # 64 ffn tiles
EPS = 1e-5
NEG = -1e4

# ---------------------------------------------------------------------------
# Workaround for this container's walrus build: it supports only ONE sync
# wait per instruction.  Rewrite the BIR just before walrus: an instruction
# with N>1 waits gets N-1 same-engine NoOps inserted before it, each carrying
# one wait.
# ---------------------------------------------------------------------------


def _split_multiwaits(bir_bytes):
    bir = json.loads(bir_bytes)
    ctr = 0
    for fn in bir.get("functions", []):
        for blk in fn.get("blocks", []):
            new = []
            for inst in blk.get("instructions", []):
                si = inst.get("sync_info")
                waits = (si or {}).get("on_wait") or []
                if len(waits) > 1:
                    for w in waits[:-1]:
                        ctr += 1
                        new.append({
                            "engine": inst["engine"],
                            "ins": [],
                            "outs": [],
                            "name": f"I-mwsplit{ctr}",
                            "opcode": "NoOp",
                            "sync_info": {"on_update": [], "on_wait": [w]},
                            "text_hint": "multiwait_split",
                        })
                    si["on_wait"] = [waits[-1]]
                new.append(inst)
            blk["instructions"] = new
    return json.dumps(bir).encode()


def _patched_compile_bir_kernel(bir_json, tmpdir, neff_name="file.neff", **kw):
    if isinstance(bir_json, str):
        bir_json = bir_json.encode()
    return _orig_compile_bir_kernel(_split_multiwaits(bir_json), tmpdir,
                                    neff_name=neff_name, **kw)


def _install_patch():
    bass2jax.compile_bir_kernel = _patched_compile_bir_kernel


def r(ap):
    """View an fp32 AP as float32r (full-rate PE mode)."""
    return ap.bitcast(F32R)


# ---------------------------------------------------------------------------
# Device program
# ---------------------------------------------------------------------------


def build_nc():
    _install_patch()
    nc = bass.Bass("TRN2")

    xTq = nc.dram_tensor("xTq", (H, QR), F32, kind="ExternalInput")
    maskT = nc.dram_tensor("maskT", (S, QR), BF16, kind="ExternalInput")
    ident_d = nc.dram_tensor("ident_d", (P, P), BF16, kind="ExternalInput")
    ones_bf_d = nc.dram_tensor("ones_bf_d", (P, 1), BF16, kind="ExternalInput")
    ones_d = nc.dram_tensor("ones_d", (P, 1), F32, kind="ExternalInput")
    ones_r_d = nc.dram_tensor("ones_r_d", (1, P), F32, kind="ExternalInput")
    wq_t = nc.dram_tensor("wq_t", (HT, P, HT, P), BF16, kind="ExternalInput")
    wk_t = nc.dram_tensor("wk_t", (HT, P, HT, P), BF16, kind="ExternalInput")
    wvT = nc.dram_tensor("wvT", (H, H), BF16, kind="ExternalInput")
    wo_t = nc.dram_tensor("wo_t", (HT, P, HT, P), BF16, kind="ExternalInput")
    w1_t = nc.dram_tensor("w1_t", (FT, P, HT, P), BF16, kind="ExternalInput")
    w2_t = nc.dram_tensor("w2_t", (HT, P, FT, P), BF16, kind="ExternalInput")
    bq = nc.dram_tensor("bq", (H,), F32, kind="ExternalInput")
    bk = nc.dram_tensor("bk", (H,), F32, kind="ExternalInput")
    bv = nc.dram_tensor("bv", (H,), F32, kind="ExternalInput")
    bwo = nc.dram_tensor("bwo", (H,), F32, kind="ExternalInput")
    b1 = nc.dram_tensor("b1", (FF,), F32, kind="ExternalInput")
    b2 = nc.dram_tensor("b2", (H,), F32, kind="ExternalInput")
    outT = nc.dram_tensor("outT", (H, QR), F32, kind="ExternalOutput")

    with tile.TileContext(nc) as tc:
        cm_const = tc.tile_pool(name="const", bufs=1)
        const = cm_const.__enter__()
        ones = const.tile([P, 1], F32, tag="ones")
        nc.sync.dma_start(r(ones[:]), r(ones_d[:]))
        ones_row = const.tile([1, P], F32, tag="ones_row")
        nc.sync.dma_start(r(ones_row[:]), r(ones_r_d[:]))
        ident = const.tile([P, P], BF16, tag="ident")
        nc.sync.dma_start(ident[:], ident_d[:])
        ones_bf = const.tile([P, 1], BF16, tag="ones_bf")
        nc.sync.dma_start(ones_bf[:], ones_bf_d[:])

        def bias_tile(name, dram_t, ntiles):
            t = const.tile([P, ntiles], F32, tag=f"b_{name}")
            nc.sync.dma_start(t[:], dram_t.rearrange("(t p) -> p t", p=P))
            return t

        bq_t = bias_tile("bq", bq, HT)
        bk_t = bias_tile("bk", bk, HT)
        bv_t = bias_tile("bv", bv, HT)
        bwo_t = bias_tile("bwo", bwo, HT)
        b1_t = bias_tile("b1", b1, FT)
        b2_t = bias_tile("b2", b2, HT)

        cm_dram = tc.tile_pool(name="dram", bufs=1, space="DRAM")
        dram = cm_dram.__enter__()
        k_own = dram.tile([H, QR], BF16, tag="k_own")
        v_own = dram.tile([QR, H], BF16, tag="v_own")
        k_gath = dram.tile([4, H, QR], BF16, tag="k_gath")
        v_gath = dram.tile([4, QR, H], BF16, tag="v_gath")
        GROUPS = [[0, 1, 2, 3], [4, 5, 6, 7]]

        # q_res / av_res / h_res live across phases; h_res reuses the
        # q_res buffer (same tag, bufs=1) once attention has consumed q.
        cm_per = tc.tile_pool(name="persist", bufs=1)
        per_p = cm_per.__enter__()
        q_res = per_p.tile([P, HT, QR], BF16, tag="q", name="q_res")
        av_res = per_p.tile([P, HT, QR], BF16, tag="av", name="av_res")

        # ============ Phase A: LN1 (own rows) + K/Q/V proj + AllGather =====
        # Each core normalizes and projects only its own 512 rows; K and V
        # are then AllGathered across the 4-core batch group.  a_own holds
        # LN1(x_own) in bf16; psum accumulation stays fp32.
        cm_ares = tc.tile_pool(name="ares", bufs=1)
        arp = cm_ares.__enter__()
        a_own = arp.tile([P, HT, QR], BF16, tag="a_own")

        def ln_chunk(lp, sps, bps, stage, dst3d, dst_sl, src_dram, src_sl, n):
            """LN a chunk of n cols: DMA f32 into `stage` [P,HT,n], compute
            stats, write the normalized result to dst3d[:, :, dst_sl] (bf16).
            """
            for i in range(HT):
                nc.sync.dma_start(r(stage[:, i, :]),
                                  r(src_dram[P * i:P * (i + 1), src_sl]))
            mean_ps = sps.tile([1, n], F32, tag="ln_mps", name="ln_mps")
            sq_ps = sps.tile([1, n], F32, tag="ln_sps", name="ln_sps")
            for i in range(HT):
                xsq = lp.tile([P, n], F32, tag="ln_sq", name="ln_sq")
                nc.scalar.activation(r(xsq[:]), stage[:, i, :], AF.Square)
                nc.tensor.matmul(mean_ps[:], r(ones[:]), r(stage[:, i, :]),
                                 start=(i == 0), stop=(i == HT - 1))
                nc.tensor.matmul(sq_ps[:], r(ones[:]), r(xsq[:]),
                                 start=(i == 0), stop=(i == HT - 1))
            mean = lp.tile([1, n], F32, tag="ln_mean", name="ln_mean")
            msq = lp.tile([1, n], F32, tag="ln_msq", name="ln_msq")
            rstd = lp.tile([1, n], F32, tag="ln_rstd", name="ln_rstd")
            nc.scalar.activation(r(mean[:]), mean_ps[:], AF.Copy, scale=1.0 / H)
            nc.scalar.activation(msq[:], sq_ps[:], AF.Copy, scale=1.0 / H)
            with nc.allow_low_precision(reason="f32r is fp32 bits"):
                nc.vector.tensor_mul(r(rstd[:]), mean[:], mean[:])
            nc.vector.tensor_sub(msq[:], msq[:], rstd[:])
            nc.vector.tensor_scalar_add(msq[:], msq[:], EPS)
            nc.vector.reciprocal(msq[:], msq[:])
            nc.scalar.activation(r(rstd[:]), msq[:], AF.Sqrt)
            bmean_ps = bps.tile([P, n], F32, tag="ln_bmps", name="ln_bmps")
            nc.tensor.matmul(bmean_ps[:], r(ones_row[:]), r(mean[:]),
                             start=True, stop=True)
            bmean = lp.tile([P, n], F32, tag="ln_bmean", name="ln_bmean")
            nc.scalar.activation(bmean[:], bmean_ps[:], AF.Copy)
            brstd_ps = bps.tile([P, n], F32, tag="ln_brps", name="ln_brps")
            nc.tensor.matmul(brstd_ps[:], r(ones_row[:]), r(rstd[:]),
                             start=True, stop=True)
            brstd = lp.tile([P, n], F32, tag="ln_brstd", name="ln_brstd")
            nc.scalar.activation(brstd[:], brstd_ps[:], AF.Copy)
            for i in range(HT):
                t1 = lp.tile([P, n], F32, tag="ln_t1", name="ln_t1")
                nc.vector.tensor_sub(t1[:], stage[:, i, :], bmean[:])
                nc.vector.tensor_mul(dst3d[:, i, dst_sl], t1[:], brstd[:])

        with tc.tile_pool(name="ln1", bufs=1) as lp, \
             tc.tile_pool(name="lnstage", bufs=2) as stp, \
             tc.tile_pool(name="ln1ps", bufs=1, space="PSUM") as lnps, \
             tc.tile_pool(name="kqproj", bufs=2) as kp, \
             tc.tile_pool(name="kqps", bufs=2, space="PSUM") as kps:
            LC = 256
            for c in range(2):
                st = stp.tile([P, HT, LC], F32, tag="lnst", name="lnst")
                ln_chunk(lp, lnps, lnps, st, a_own,
                         slice(LC * c, LC * (c + 1)),
                         xTq, slice(LC * c, LC * (c + 1)), LC)
            # K proj (own rows)
            for dM in range(HT):
                kw = kp.tile([P, HT, P], BF16, tag="kw", name=f"kw{dM}")
                nc.sync.dma_start(kw[:], wk_t[dM])
                ps = kps.tile([P, QR], F32, tag="kp", name="kp")
                for ht in range(HT):
                    nc.tensor.matmul(ps[:], kw[:, ht, :], a_own[:, ht, :],
                                     start=(ht == 0), stop=(ht == HT - 1))
                kst = kp.tile([P, QR], BF16, tag="kst", name="kst")
                nc.scalar.activation(kst[:], ps[:], AF.Identity,
                                     bias=bk_t[:, dM:dM + 1])
                nc.sync.dma_start(k_own[P * dM:P * (dM + 1), :], kst[:])
            nc.gpsimd.collective_compute(
                "AllGather", OP.bypass, replica_groups=GROUPS,
                ins=[k_own[:]], outs=[k_gath[:]])
            # Q proj (own rows)
            for dM in range(HT):
                qw = kp.tile([P, HT, P], BF16, tag="kw", name=f"qw{dM}")
                nc.sync.dma_start(qw[:], wq_t[dM])
                ps = kps.tile([P, QR], F32, tag="kp", name="qp")
                for ht in range(HT):
                    nc.tensor.matmul(ps[:], qw[:, ht, :], a_own[:, ht, :],
                                     start=(ht == 0), stop=(ht == HT - 1))
                nc.scalar.activation(q_res[:, dM, :], ps[:], AF.Identity,
                                     bias=bq_t[:, dM:dM + 1])
            # V proj (own rows, seq-major output)
            for dc in range(4):
                wv_c = kp.tile([P, HT, 512], BF16, tag="wv_c",
                               name=f"wv{dc}")
                nc.sync.dma_start(
                    wv_c[:], wvT.rearrange("(t p) d -> p t d", p=P)
                    [:, :, 512 * dc:512 * (dc + 1)])
                for sb in range(4):
                    ps = kps.tile([P, 512], F32, tag="kp", name="vp")
                    for ht in range(HT):
                        nc.tensor.matmul(
                            ps[:], a_own[:, ht, P * sb:P * (sb + 1)],
                            wv_c[:, ht, :],
                            start=(ht == 0), stop=(ht == HT - 1))
                    vst = kp.tile([P, 512], BF16, tag="kst", name="vst")
                    nc.scalar.activation(vst[:], ps[:], AF.Copy)
                    nc.sync.dma_start(
                        v_own[P * sb:P * (sb + 1), 512 * dc:512 * (dc + 1)],
                        vst[:])
            nc.gpsimd.collective_compute(
                "AllGather", OP.bypass, replica_groups=GROUPS,
                ins=[v_own[:]], outs=[v_gath[:]])
        cm_ares.__exit__(None, None, None)

        # ============ Phase B: attention (pipelined across heads) ==========
        with tc.tile_pool(name="attn", bufs=2) as ap_, \
             tc.tile_pool(name="attn1", bufs=1) as ap1, \
             tc.tile_pool(name="attnps", bufs=2, space="PSUM") as aps:
            mask_res = ap1.tile([P, HT, QR], BF16, tag="mask_res")
            nc.sync.dma_start(mask_res[:],
                              maskT.rearrange("(t p) s -> p t s", p=P))
            kh_t = [None] * NH
            vh_t = [None] * NH
            pt_t = [None] * NH

            def load_head(h):
                kh_t[h] = ap_.tile([P, S], BF16, tag="kh", name=f"kh{h}")
                vh_t[h] = ap_.tile([P, HT, P], BF16, tag="vh", name=f"vh{h}")
                for j in range(4):
                    nc.sync.dma_start(kh_t[h][:, QR * j:QR * (j + 1)],
                                      k_gath[j][P * h:P * (h + 1), :])
                    nc.sync.dma_start(
                        vh_t[h][:, 4 * j:4 * (j + 1), :],
                        v_gath[j].rearrange("(t p) d -> p t d", p=P)
                        [:, :, P * h:P * (h + 1)])

            def scores_exp(h):
                pt_t[h] = ap_.tile([P, HT, QR], BF16, tag="pt", name=f"pt{h}")
                pt = pt_t[h]
                for kb in range(HT):
                    sp = aps.tile([P, QR], F32, tag="sp")
                    nc.tensor.matmul(sp[:], kh_t[h][:, P * kb:P * (kb + 1)],
                                     q_res[:, h, :], start=True, stop=False)
                    nc.tensor.matmul(sp[:], ident[:], mask_res[:, kb, :],
                                     start=False, stop=True)
                    nc.scalar.activation(pt[:, kb, :], sp[:], AF.Exp)
                kh_t[h] = None

            def denom_av(h):
                pt = pt_t[h]
                dn = aps.tile([1, QR], F32, tag="dn")
                for kb in range(HT):
                    nc.tensor.matmul(dn[:], ones_bf[:], pt[:, kb, :],
                                     start=(kb == 0), stop=(kb == HT - 1))
                rec = ap_.tile([1, QR], F32, tag="rec")
                with nc.allow_low_precision(reason="f32r is fp32 bits"):
                    nc.vector.reciprocal(r(rec[:]), dn[:])
                brec_ps = aps.tile([P, QR], F32, tag="brec_ps")
                nc.tensor.matmul(brec_ps[:], r(ones_row[:]), r(rec[:]),
                                 start=True, stop=True)
                brec = ap_.tile([P, QR], F32, tag="brec")
                nc.scalar.activation(brec[:], brec_ps[:], AF.Copy)
                avp = aps.tile([P, QR], F32, tag="avp")
                for kb in range(HT):
                    nc.tensor.matmul(avp[:], vh_t[h][:, kb, :],
                                     pt[:, kb, :],
                                     start=(kb == 0), stop=(kb == HT - 1))
                avn = ap_.tile([P, QR], F32, tag="avn", name=f"avn{h}")
                nc.vector.tensor_mul(avn[:], avp[:], brec[:])
                nc.vector.tensor_scalar_add(av_res[:, h, :], avn[:],
                                            bv_t[:, h:h + 1])
                vh_t[h] = None
                pt_t[h] = None

            load_head(0)
            for h in range(NH):
                if h + 1 < NH:
                    load_head(h + 1)
                scores_exp(h)
                if h > 0:
                    denom_av(h - 1)
            denom_av(NH - 1)

        # ============ Phase C: WO + residual + LN2 (stats interleaved) =====
        h_res = per_p.tile([P, HT, QR], F32, tag="h", name="h_res")
        cm_f = tc.tile_pool(name="fres", bufs=1)
        f_p = cm_f.__enter__()
        f_res = f_p.tile([P, FT, QR], BF16, tag="f_res")
        cm_g = tc.tile_pool(name="gres", bufs=1)
        g_p = cm_g.__enter__()
        g_res = g_p.tile([P, HT, QR], BF16, tag="g_res")
        with tc.tile_pool(name="wo", bufs=2) as wop, \
             tc.tile_pool(name="wops", bufs=2, space="PSUM") as wops, \
             tc.tile_pool(name="ln2", bufs=1) as l2p, \
             tc.tile_pool(name="ln2ps", bufs=1, space="PSUM") as l2ps:
            mean_ps = l2ps.tile([1, QR], F32, tag="ln2_mps")
            sq_ps = l2ps.tile([1, QR], F32, tag="ln2_sps")
            for dM in range(HT):
                wot = wop.tile([P, HT, P], BF16, tag="wot")
                nc.sync.dma_start(wot[:], wo_t[dM])
                xq_t = wop.tile([P, QR], F32, tag="xq_t")
                nc.sync.dma_start(xq_t[:], xTq[P * dM:P * (dM + 1), :])
                ps = wops.tile([P, QR], F32, tag="wopsum")
                for ht in range(HT):
                    nc.tensor.matmul(ps[:], wot[:, ht, :],
                                     av_res[:, ht, :],
                                     start=(ht == 0), stop=(ht == HT - 1))
                with nc.allow_low_precision(reason="f32r is fp32 bits"):
                    nc.vector.scalar_tensor_tensor(
                        r(h_res[:, dM, :]), ps[:], bwo_t[:, dM:dM + 1],
                        xq_t[:], op0=OP.add, op1=OP.add)
                xsq = l2p.tile([P, QR], F32, tag="ln2_sq", name="ln2_sq")
                nc.scalar.activation(r(xsq[:]), h_res[:, dM, :], AF.Square)
                nc.tensor.matmul(mean_ps[:], r(ones[:]), r(h_res[:, dM, :]),
                                 start=(dM == 0), stop=(dM == HT - 1))
                nc.tensor.matmul(sq_ps[:], r(ones[:]), r(xsq[:]),
                                 start=(dM == 0), stop=(dM == HT - 1))
            mean = l2p.tile([1, QR], F32, tag="ln2_mean")
            msq = l2p.tile([1, QR], F32, tag="ln2_msq")
            rstd = l2p.tile([1, QR], F32, tag="ln2_rstd")
            nc.scalar.activation(r(mean[:]), mean_ps[:], AF.Copy, scale=1.0 / H)
            nc.scalar.activation(msq[:], sq_ps[:], AF.Copy, scale=1.0 / H)
            with nc.allow_low_precision(reason="f32r is fp32 bits"):
                nc.vector.tensor_mul(r(rstd[:]), mean[:], mean[:])
            nc.vector.tensor_sub(msq[:], msq[:], rstd[:])
            nc.vector.tensor_scalar_add(msq[:], msq[:], EPS)
            nc.vector.reciprocal(msq[:], msq[:])
            nc.scalar.activation(r(rstd[:]), msq[:], AF.Sqrt)
            bmean_ps = wops.tile([P, QR], F32, tag="wopsum", name="l2bm")
            nc.tensor.matmul(bmean_ps[:], r(ones_row[:]), r(mean[:]),
                             start=True, stop=True)
            bmean = l2p.tile([P, QR], F32, tag="ln2_bmean")
            nc.scalar.activation(bmean[:], bmean_ps[:], AF.Copy)
            brstd_ps = wops.tile([P, QR], F32, tag="wopsum", name="l2br")
            nc.tensor.matmul(brstd_ps[:], r(ones_row[:]), r(rstd[:]),
                             start=True, stop=True)
            brstd = l2p.tile([P, QR], F32, tag="ln2_brstd")
            nc.scalar.activation(brstd[:], brstd_ps[:], AF.Copy)
            for i in range(HT):
                t1 = l2p.tile([P, QR], F32, tag="ln2_t1", name="ln2_t1")
                nc.vector.tensor_sub(t1[:], h_res[:, i, :], bmean[:])
                nc.vector.tensor_mul(g_res[:, i, :], t1[:], brstd[:])

        with tc.tile_pool(name="ffn1", bufs=3) as f1p, \
             tc.tile_pool(name="f1ps", bufs=2, space="PSUM") as f1ps:
            for fM in range(FT):
                w1t = f1p.tile([P, HT, P], BF16, tag="w1t")
                nc.sync.dma_start(w1t[:], w1_t[fM])
                ps = f1ps.tile([P, QR], F32, tag="f1psum")
                for ht in range(HT):
                    nc.tensor.matmul(ps[:], w1t[:, ht, :], g_res[:, ht, :],
                                     start=(ht == 0), stop=(ht == HT - 1))
                nc.scalar.activation(f_res[:, fM, :], ps[:], AF.Gelu,
                                     bias=b1_t[:, fM:fM + 1])
        cm_g.__exit__(None, None, None)

        with tc.tile_pool(name="ffn2", bufs=3) as f2p, \
             tc.tile_pool(name="f2ps", bufs=2, space="PSUM") as f2ps:
            for dM in range(HT):
                ps = f2ps.tile([P, QR], F32, tag="f2psum")
                for q4 in range(4):
                    w2t = f2p.tile([P, HT, P], BF16, tag="w2t")
                    nc.sync.dma_start(
                        w2t[:], w2_t[dM][:, 16 * q4:16 * (q4 + 1), :])
                    for ft in range(HT):
                        kk = 16 * q4 + ft
                        nc.tensor.matmul(ps[:], w2t[:, ft, :],
                                         f_res[:, kk, :],
                                         start=(kk == 0), stop=(kk == FT - 1))
                ost = f2p.tile([P, QR], F32, tag="ost")
                nc.vector.scalar_tensor_tensor(
                    ost[:], ps[:], b2_t[:, dM:dM + 1], h_res[:, dM, :],
                    op0=OP.add, op1=OP.add)
                nc.sync.dma_start(outT[P * dM:P * (dM + 1), :], ost[:])
        cm_f.__exit__(None, None, None)
        cm_per.__exit__(None, None, None)
        cm_dram.__exit__(None, None, None)
        cm_const.__exit__(None, None, None)

    return nc


# ---------------------------------------------------------------------------
# Host side
# ---------------------------------------------------------------------------

_CACHE = {}


def _get_nc():
    if "nc" not in _CACHE:
        _CACHE["nc"] = build_nc()
    return _CACHE["nc"]


def make_in_maps(inputs):
    import ml_dtypes
    BF = ml_dtypes.bfloat16
    x = np.asarray(inputs["x"], np.float32)
    scale = np.float32(1.0 / np.sqrt(HD))
    wqkv = np.asarray(inputs["wqkv_w"], np.float32)
    wqkv_b = np.asarray(inputs["wqkv_b"], np.float32)
    ln1w = np.asarray(inputs["ln1_w"], np.float32)
    ln1b = np.asarray(inputs["ln1_b"], np.float32)
    ln2w = np.asarray(inputs["ln2_w"], np.float32)
    ln2b = np.asarray(inputs["ln2_b"], np.float32)
    w1 = np.asarray(inputs["w1"], np.float32)
    wq, wk, wv = wqkv[:H], wqkv[H:2 * H], wqkv[2 * H:]

    def tile_kxm(wT, dt=np.float32):
        # [K, M] -> [mM, p, kt, m2] so each [128, kt*128] lhsT load is
        # contiguous per partition
        K_, M_ = wT.shape
        return np.ascontiguousarray(
            wT.reshape(K_ // P, P, M_ // P, P).transpose(2, 1, 0, 3)).astype(dt)

    # LN affine params are folded into the downstream projections:
    # W @ (y*w + b) = (W*diag(w)) @ y + W @ b   (exact algebra)
    shared = {
        "ones_d": np.ones((P, 1), np.float32),
        "ones_bf_d": np.ones((P, 1), BF),
        "ones_r_d": np.ones((1, P), np.float32),
        "ident_d": np.eye(P, dtype=np.float32).astype(BF),
        "wq_t": tile_kxm(wq.T * ln1w[:, None] * scale, BF),
        "wk_t": tile_kxm(wk.T * ln1w[:, None], BF),
        "wvT": np.ascontiguousarray(wv.T * ln1w[:, None]).astype(BF),
        "wo_t": tile_kxm(np.asarray(inputs["wo_w"], np.float32).T, BF),
        "w1_t": tile_kxm(w1.T * ln2w[:, None], BF),
        "w2_t": tile_kxm(np.asarray(inputs["w2"], np.float32).T, BF),
        "bq": np.ascontiguousarray((wqkv_b[:H] + wq @ ln1b) * scale),
        "bk": np.ascontiguousarray(wqkv_b[H:2 * H] + wk @ ln1b),
        "bv": np.ascontiguousarray(wqkv_b[2 * H:] + wv @ ln1b),
        "bwo": np.asarray(inputs["wo_b"], np.float32),
        "b1": np.asarray(inputs["b1"], np.float32) + w1 @ ln2b,
        "b2": np.asarray(inputs["b2"], np.float32),
    }
    kidx = np.arange(S)
    in_maps = []
    for core in range(8):
        b, c = divmod(core, 4)
        q0 = QR * c
        qidx = q0 + np.arange(QR)
        m = np.where(kidx[:, None] <= qidx[None, :], np.float32(0),
                     np.float32(NEG)).astype(BF)
        in_maps.append(dict(
            shared,
            xTq=np.ascontiguousarray(x[b, q0:q0 + QR].T),
            maskT=np.ascontiguousarray(m),
        ))
    return in_maps


def run_cores(inputs, **run_kw):
    nc = _get_nc()
    in_maps = make_in_maps(inputs)
    return nc, run_bass_kernel_spmd(nc, in_maps, core_ids=list(range(8)),
                                    **run_kw)


def kernel(**inputs):
    _, res = run_cores(inputs)
    out = np.empty((B, S, H), np.float32)
    for core in range(8):
        b, c = divmod(core, 4)
        out[b, QR * c:QR * (c + 1), :] = res.results[core]["outT"].T
    return out


# revision 21
# speedup vs baseline: 1.5359x; 1.0172x over previous
"""Trainium2 Bass kernel for a dense transformer decoder block.

Problem: B=2, S=2048, H=2048, NH=16 (head_dim=128), FFN=8192, fp32.

Sharding (zero collectives): 8 cores = 2 batches x 4 query-chunks of 512
contiguous rows.  Every core redundantly computes LN1 + K/V projections for
its full batch, then attention / WO / LN2 / FFN for its own 512 rows.  The
output is disjoint across cores; the host concatenates shards.

v2 layout: feature-major on device.  LN1 is chunked over the sequence so its
vector work overlaps K-projection matmuls; `a` (LN1 output) stays resident in
SBUF for both K and V projections; the causal mask is added on the PE via an
identity-lhsT accumulation matmul so softmax is a pure mm->exp chain;
attention is software-pipelined across heads; `h` (post-WO residual) stays
resident in SBUF; the FFN runs in bf16 (same PE rate, half the DMA).
"""

import json

import numpy as np

import concourse.bass as bass
import concourse.bass2jax as bass2jax
import concourse.mybir as mybir
import concourse.tile as tile
from concourse.bass_utils import compile_bir_kernel as _orig_compile_bir_kernel
from concourse.bass_utils import run_bass_kernel_spmd

F32 = mybir.dt.float32
F32R = mybir.dt.float32r
BF16 = mybir.dt.bfloat16
AF = mybir.ActivationFunctionType
OP = mybir.AluOpType

B, S, H, NH, HD, FF = 2, 2048, 2048, 16, 128, 8192
P = 128
QR = 512            # query rows per core
HT = H // P         # 16 feature tiles
FT = FF // P        # 64 ffn tiles
EPS = 1e-5
NEG = -1e4

# ---------------------------------------------------------------------------
# Workaround for this container's walrus build: it supports only ONE sync
# wait per instruction.  Rewrite the BIR just before walrus: an instruction
# with N>1 waits gets N-1 same-engine NoOps inserted before it, each carrying
# one wait.
# ---------------------------------------------------------------------------


def _split_multiwaits(bir_bytes):
    bir = json.loads(bir_bytes)
    ctr = 0
    for fn in bir.get("functions", []):
        for blk in fn.get("blocks", []):
            new = []
            for inst in blk.get("instructions", []):
                si = inst.get("sync_info")
                waits = (si or {}).get("on_wait") or []
                if len(waits) > 1:
                    for w in waits[:-1]:
                        ctr += 1
                        new.append({
                            "engine": inst["engine"],
                            "ins": [],
                            "outs": [],
                            "name": f"I-mwsplit{ctr}",
                            "opcode": "NoOp",
                            "sync_info": {"on_update": [], "on_wait": [w]},
                            "text_hint": "multiwait_split",
                        })
                    si["on_wait"] = [waits[-1]]
                new.append(inst)
            blk["instructions"] = new
    return json.dumps(bir).encode()


def _patched_compile_bir_kernel(bir_json, tmpdir, neff_name="file.neff", **kw):
    if isinstance(bir_json, str):
        bir_json = bir_json.encode()
    return _orig_compile_bir_kernel(_split_multiwaits(bir_json), tmpdir,
                                    neff_name=neff_name, **kw)


def _install_patch():
    bass2jax.compile_bir_kernel = _patched_compile_bir_kernel


def r(ap):
    """View an fp32 AP as float32r (full-rate PE mode)."""
    return ap.bitcast(F32R)


# ---------------------------------------------------------------------------
# Device program
# ---------------------------------------------------------------------------


def build_nc():
    _install_patch()
    nc = bass.Bass("TRN2")

    xTq = nc.dram_tensor("xTq", (H, QR), F32, kind="ExternalInput")
    maskT = nc.dram_tensor("maskT", (S, QR), BF16, kind="ExternalInput")
    ident_d = nc.dram_tensor("ident_d", (P, P), BF16, kind="ExternalInput")
    ones_bf_d = nc.dram_tensor("ones_bf_d", (P, 1), BF16, kind="ExternalInput")
    ones_d = nc.dram_tensor("ones_d", (P, 1), F32, kind="ExternalInput")
    ones_r_d = nc.dram_tensor("ones_r_d", (1, P), F32, kind="ExternalInput")
    wq_t = nc.dram_tensor("wq_t", (HT, P, HT, P), BF16, kind="ExternalInput")
    wk_t = nc.dram_tensor("wk_t", (HT, P, HT, P), BF16, kind="ExternalInput")
    wvT = nc.dram_tensor("wvT", (H, H), BF16, kind="ExternalInput")
    wo_t = nc.dram_tensor("wo_t", (HT, P, HT, P), BF16, kind="ExternalInput")
    w1_t = nc.dram_tensor("w1_t", (FT, P, HT, P), BF16, kind="ExternalInput")
    w2_t = nc.dram_tensor("w2_t", (HT, P, FT, P), BF16, kind="ExternalInput")
    bq = nc.dram_tensor("bq", (H,), F32, kind="ExternalInput")
    bk = nc.dram_tensor("bk", (H,), F32, kind="ExternalInput")
    bv = nc.dram_tensor("bv", (H,), F32, kind="ExternalInput")
    bwo = nc.dram_tensor("bwo", (H,), F32, kind="ExternalInput")
    b1 = nc.dram_tensor("b1", (FF,), F32, kind="ExternalInput")
    b2 = nc.dram_tensor("b2", (H,), F32, kind="ExternalInput")
    outT = nc.dram_tensor("outT", (H, QR), F32, kind="ExternalOutput")

    with tile.TileContext(nc) as tc:
        cm_const = tc.tile_pool(name="const", bufs=1)
        const = cm_const.__enter__()
        ones = const.tile([P, 1], F32, tag="ones")
        nc.sync.dma_start(r(ones[:]), r(ones_d[:]))
        ones_row = const.tile([1, P], F32, tag="ones_row")
        nc.sync.dma_start(r(ones_row[:]), r(ones_r_d[:]))
        ident = const.tile([P, P], BF16, tag="ident")
        nc.sync.dma_start(ident[:], ident_d[:])
        ones_bf = const.tile([P, 1], BF16, tag="ones_bf")
        nc.sync.dma_start(ones_bf[:], ones_bf_d[:])

        def bias_tile(name, dram_t, ntiles):
            t = const.tile([P, ntiles], F32, tag=f"b_{name}")
            nc.sync.dma_start(t[:], dram_t.rearrange("(t p) -> p t", p=P))
            return t

        bq_t = bias_tile("bq", bq, HT)
        bk_t = bias_tile("bk", bk, HT)
        bv_t = bias_tile("bv", bv, HT)
        bwo_t = bias_tile("bwo", bwo, HT)
        b1_t = bias_tile("b1", b1, FT)
        b2_t = bias_tile("b2", b2, HT)

        cm_dram = tc.tile_pool(name="dram", bufs=1, space="DRAM")
        dram = cm_dram.__enter__()
        k_own = [dram.tile([H // 2, QR], BF16, tag=f"k_own{i}",
                           name=f"k_own{i}") for i in range(2)]
        v_own = [dram.tile([QR, 512], BF16, tag=f"v_own{i}",
                           name=f"v_own{i}") for i in range(4)]
        k_gath = [dram.tile([4, H // 2, QR], BF16, tag=f"k_gath{i}",
                            name=f"k_gath{i}") for i in range(2)]
        v_gath = [dram.tile([4, QR, 512], BF16, tag=f"v_gath{i}",
                            name=f"v_gath{i}") for i in range(4)]
        GROUPS = [[0, 1, 2, 3], [4, 5, 6, 7]]

        # q_res / av_res / h_res live across phases; h_res reuses the
        # q_res buffer (same tag, bufs=1) once attention has consumed q.
        cm_per = tc.tile_pool(name="persist", bufs=1)
        per_p = cm_per.__enter__()
        q_res = per_p.tile([P, HT, QR], BF16, tag="q", name="q_res")
        av_res = per_p.tile([P, HT, QR], BF16, tag="av", name="av_res")

        # ============ Phase A: LN1 (own rows) + K/Q/V proj + AllGather =====
        # Each core normalizes and projects only its own 512 rows; K and V
        # are then AllGathered across the 4-core batch group.  a_own holds
        # LN1(x_own) in bf16; psum accumulation stays fp32.
        cm_ares = tc.tile_pool(name="ares", bufs=1)
        arp = cm_ares.__enter__()
        a_own = arp.tile([P, HT, QR], BF16, tag="a_own")

        def ln_chunk(lp, sps, bps, stage, dst3d, dst_sl, src_dram, src_sl, n):
            """LN a chunk of n cols: DMA f32 into `stage` [P,HT,n], compute
            stats, write the normalized result to dst3d[:, :, dst_sl] (bf16).
            """
            for i in range(HT):
                nc.sync.dma_start(r(stage[:, i, :]),
                                  r(src_dram[P * i:P * (i + 1), src_sl]))
            mean_ps = sps.tile([1, n], F32, tag="ln_mps", name="ln_mps")
            sq_ps = sps.tile([1, n], F32, tag="ln_sps", name="ln_sps")
            for i in range(HT):
                xsq = lp.tile([P, n], F32, tag="ln_sq", name="ln_sq")
                nc.scalar.activation(r(xsq[:]), stage[:, i, :], AF.Square)
                nc.tensor.matmul(mean_ps[:], r(ones[:]), r(stage[:, i, :]),
                                 start=(i == 0), stop=(i == HT - 1))
                nc.tensor.matmul(sq_ps[:], r(ones[:]), r(xsq[:]),
                                 start=(i == 0), stop=(i == HT - 1))
            mean = lp.tile([1, n], F32, tag="ln_mean", name="ln_mean")
            msq = lp.tile([1, n], F32, tag="ln_msq", name="ln_msq")
            rstd = lp.tile([1, n], F32, tag="ln_rstd", name="ln_rstd")
            nc.scalar.activation(r(mean[:]), mean_ps[:], AF.Copy, scale=1.0 / H)
            nc.scalar.activation(msq[:], sq_ps[:], AF.Copy, scale=1.0 / H)
            with nc.allow_low_precision(reason="f32r is fp32 bits"):
                nc.vector.tensor_mul(r(rstd[:]), mean[:], mean[:])
            nc.vector.tensor_sub(msq[:], msq[:], rstd[:])
            nc.vector.tensor_scalar_add(msq[:], msq[:], EPS)
            nc.vector.reciprocal(msq[:], msq[:])
            nc.scalar.activation(r(rstd[:]), msq[:], AF.Sqrt)
            bmean_ps = bps.tile([P, n], F32, tag="ln_bmps", name="ln_bmps")
            nc.tensor.matmul(bmean_ps[:], r(ones_row[:]), r(mean[:]),
                             start=True, stop=True)
            bmean = lp.tile([P, n], F32, tag="ln_bmean", name="ln_bmean")
            nc.scalar.activation(bmean[:], bmean_ps[:], AF.Copy)
            brstd_ps = bps.tile([P, n], F32, tag="ln_brps", name="ln_brps")
            nc.tensor.matmul(brstd_ps[:], r(ones_row[:]), r(rstd[:]),
                             start=True, stop=True)
            brstd = lp.tile([P, n], F32, tag="ln_brstd", name="ln_brstd")
            nc.scalar.activation(brstd[:], brstd_ps[:], AF.Copy)
            for i in range(HT):
                t1 = lp.tile([P, n], F32, tag="ln_t1", name="ln_t1")
                nc.vector.tensor_sub(t1[:], stage[:, i, :], bmean[:])
                nc.vector.tensor_mul(dst3d[:, i, dst_sl], t1[:], brstd[:])

        with tc.tile_pool(name="ln1", bufs=1) as lp, \
             tc.tile_pool(name="lnstage", bufs=2) as stp, \
             tc.tile_pool(name="ln1ps", bufs=1, space="PSUM") as lnps, \
             tc.tile_pool(name="kqproj", bufs=2) as kp, \
             tc.tile_pool(name="kqps", bufs=2, space="PSUM") as kps:
            LC = 256
            for c in range(2):
                st = stp.tile([P, HT, LC], F32, tag="lnst", name="lnst")
                ln_chunk(lp, lnps, lnps, st, a_own,
                         slice(LC * c, LC * (c + 1)),
                         xTq, slice(LC * c, LC * (c + 1)), LC)
            # K proj (own rows); gather each feature half as it completes
            for dM in range(HT):
                kw = kp.tile([P, HT, P], BF16, tag="kw", name=f"kw{dM}")
                nc.sync.dma_start(kw[:], wk_t[dM])
                ps = kps.tile([P, QR], F32, tag="kp", name="kp")
                for ht in range(HT):
                    nc.tensor.matmul(ps[:], kw[:, ht, :], a_own[:, ht, :],
                                     start=(ht == 0), stop=(ht == HT - 1))
                kst = kp.tile([P, QR], BF16, tag="kst", name="kst")
                nc.scalar.activation(kst[:], ps[:], AF.Identity,
                                     bias=bk_t[:, dM:dM + 1])
                nc.sync.dma_start(
                    k_own[dM // 8][P * (dM % 8):P * (dM % 8 + 1), :], kst[:])
                if dM % 8 == 7:
                    nc.gpsimd.collective_compute(
                        "AllGather", OP.bypass, replica_groups=GROUPS,
                        ins=[k_own[dM // 8][:]], outs=[k_gath[dM // 8][:]])
            # Q proj (own rows)
            for dM in range(HT):
                qw = kp.tile([P, HT, P], BF16, tag="kw", name=f"qw{dM}")
                nc.sync.dma_start(qw[:], wq_t[dM])
                ps = kps.tile([P, QR], F32, tag="kp", name="qp")
                for ht in range(HT):
                    nc.tensor.matmul(ps[:], qw[:, ht, :], a_own[:, ht, :],
                                     start=(ht == 0), stop=(ht == HT - 1))
                nc.scalar.activation(q_res[:, dM, :], ps[:], AF.Identity,
                                     bias=bq_t[:, dM:dM + 1])
            # V proj (own rows, seq-major output)
            for dc in range(4):
                wv_c = kp.tile([P, HT, 512], BF16, tag="wv_c",
                               name=f"wv{dc}")
                nc.sync.dma_start(
                    wv_c[:], wvT.rearrange("(t p) d -> p t d", p=P)
                    [:, :, 512 * dc:512 * (dc + 1)])
                for sb in range(4):
                    ps = kps.tile([P, 512], F32, tag="kp", name="vp")
                    for ht in range(HT):
                        nc.tensor.matmul(
                            ps[:], a_own[:, ht, P * sb:P * (sb + 1)],
                            wv_c[:, ht, :],
                            start=(ht == 0), stop=(ht == HT - 1))
                    vst = kp.tile([P, 512], BF16, tag="kst", name="vst")
                    nc.scalar.activation(vst[:], ps[:], AF.Copy)
                    nc.sync.dma_start(v_own[dc][P * sb:P * (sb + 1), :],
                                      vst[:])
                nc.gpsimd.collective_compute(
                    "AllGather", OP.bypass, replica_groups=GROUPS,
                    ins=[v_own[dc][:]], outs=[v_gath[dc][:]])
        cm_ares.__exit__(None, None, None)

        # ============ Phase B: attention (pipelined across heads) ==========
        with tc.tile_pool(name="attn", bufs=2) as ap_, \
             tc.tile_pool(name="attn1", bufs=1) as ap1, \
             tc.tile_pool(name="attnps", bufs=2, space="PSUM") as aps:
            mask_res = ap1.tile([P, HT, QR], BF16, tag="mask_res")
            nc.sync.dma_start(mask_res[:],
                              maskT.rearrange("(t p) s -> p t s", p=P))
            kh_t = [None] * NH
            vh_t = [None] * NH
            pt_t = [None] * NH

            def load_head(h):
                kh_t[h] = ap_.tile([P, S], BF16, tag="kh", name=f"kh{h}")
                vh_t[h] = ap_.tile([P, HT, P], BF16, tag="vh", name=f"vh{h}")
                for j in range(4):
                    nc.sync.dma_start(
                        kh_t[h][:, QR * j:QR * (j + 1)],
                        k_gath[h // 8][j][P * (h % 8):P * (h % 8 + 1), :])
                    nc.sync.dma_start(
                        vh_t[h][:, 4 * j:4 * (j + 1), :],
                        v_gath[h // 4][j].rearrange("(t p) d -> p t d", p=P)
                        [:, :, P * (h % 4):P * (h % 4 + 1)])

            def scores_exp(h):
                pt_t[h] = ap_.tile([P, HT, QR], BF16, tag="pt", name=f"pt{h}")
                pt = pt_t[h]
                for kb in range(HT):
                    sp = aps.tile([P, QR], F32, tag="sp")
                    nc.tensor.matmul(sp[:], kh_t[h][:, P * kb:P * (kb + 1)],
                                     q_res[:, h, :], start=True, stop=False)
                    nc.tensor.matmul(sp[:], ident[:], mask_res[:, kb, :],
                                     start=False, stop=True)
                    nc.scalar.activation(pt[:, kb, :], sp[:], AF.Exp)
                kh_t[h] = None

            def denom_av(h):
                pt = pt_t[h]
                dn = aps.tile([1, QR], F32, tag="dn")
                for kb in range(HT):
                    nc.tensor.matmul(dn[:], ones_bf[:], pt[:, kb, :],
                                     start=(kb == 0), stop=(kb == HT - 1))
                rec = ap_.tile([1, QR], F32, tag="rec")
                with nc.allow_low_precision(reason="f32r is fp32 bits"):
                    nc.vector.reciprocal(r(rec[:]), dn[:])
                brec_ps = aps.tile([P, QR], F32, tag="brec_ps")
                nc.tensor.matmul(brec_ps[:], r(ones_row[:]), r(rec[:]),
                                 start=True, stop=True)
                brec = ap_.tile([P, QR], F32, tag="brec")
                nc.scalar.activation(brec[:], brec_ps[:], AF.Copy)
                avp = aps.tile([P, QR], F32, tag="avp")
                for kb in range(HT):
                    nc.tensor.matmul(avp[:], vh_t[h][:, kb, :],
                                     pt[:, kb, :],
                                     start=(kb == 0), stop=(kb == HT - 1))
                avn = ap_.tile([P, QR], F32, tag="avn", name=f"avn{h}")
                nc.vector.tensor_mul(avn[:], avp[:], brec[:])
                nc.vector.tensor_scalar_add(av_res[:, h, :], avn[:],
                                            bv_t[:, h:h + 1])
                vh_t[h] = None
                pt_t[h] = None

            load_head(0)
            for h in range(NH):
                if h + 1 < NH:
                    load_head(h + 1)
                scores_exp(h)
                if h > 0:
                    denom_av(h - 1)
            denom_av(NH - 1)

        # ============ Phase C: WO + residual + LN2 (stats interleaved) =====
        h_res = per_p.tile([P, HT, QR], F32, tag="h", name="h_res")
        cm_f = tc.tile_pool(name="fres", bufs=1)
        f_p = cm_f.__enter__()
        f_res = f_p.tile([P, FT, QR], BF16, tag="f_res")
        cm_g = tc.tile_pool(name="gres", bufs=1)
        g_p = cm_g.__enter__()
        g_res = g_p.tile([P, HT, QR], BF16, tag="g_res")
        with tc.tile_pool(name="wo", bufs=2) as wop, \
             tc.tile_pool(name="wops", bufs=2, space="PSUM") as wops, \
             tc.tile_pool(name="ln2", bufs=1) as l2p, \
             tc.tile_pool(name="ln2ps", bufs=1, space="PSUM") as l2ps:
            mean_ps = l2ps.tile([1, QR], F32, tag="ln2_mps")
            sq_ps = l2ps.tile([1, QR], F32, tag="ln2_sps")
            for dM in range(HT):
                wot = wop.tile([P, HT, P], BF16, tag="wot")
                nc.sync.dma_start(wot[:], wo_t[dM])
                xq_t = wop.tile([P, QR], F32, tag="xq_t")
                nc.sync.dma_start(xq_t[:], xTq[P * dM:P * (dM + 1), :])
                ps = wops.tile([P, QR], F32, tag="wopsum")
                for ht in range(HT):
                    nc.tensor.matmul(ps[:], wot[:, ht, :],
                                     av_res[:, ht, :],
                                     start=(ht == 0), stop=(ht == HT - 1))
                with nc.allow_low_precision(reason="f32r is fp32 bits"):
                    nc.vector.scalar_tensor_tensor(
                        r(h_res[:, dM, :]), ps[:], bwo_t[:, dM:dM + 1],
                        xq_t[:], op0=OP.add, op1=OP.add)
                xsq = l2p.tile([P, QR], F32, tag="ln2_sq", name="ln2_sq")
                nc.scalar.activation(r(xsq[:]), h_res[:, dM, :], AF.Square)
                nc.tensor.matmul(mean_ps[:], r(ones[:]), r(h_res[:, dM, :]),
                                 start=(dM == 0), stop=(dM == HT - 1))
                nc.tensor.matmul(sq_ps[:], r(ones[:]), r(xsq[:]),
                                 start=(dM == 0), stop=(dM == HT - 1))
            mean = l2p.tile([1, QR], F32, tag="ln2_mean")
            msq = l2p.tile([1, QR], F32, tag="ln2_msq")
            rstd = l2p.tile([1, QR], F32, tag="ln2_rstd")
            nc.scalar.activation(r(mean[:]), mean_ps[:], AF.Copy, scale=1.0 / H)
            nc.scalar.activation(msq[:], sq_ps[:], AF.Copy, scale=1.0 / H)
            with nc.allow_low_precision(reason="f32r is fp32 bits"):
                nc.vector.tensor_mul(r(rstd[:]), mean[:], mean[:])
            nc.vector.tensor_sub(msq[:], msq[:], rstd[:])
            nc.vector.tensor_scalar_add(msq[:], msq[:], EPS)
            nc.vector.reciprocal(msq[:], msq[:])
            nc.scalar.activation(r(rstd[:]), msq[:], AF.Sqrt)
            bmean_ps = wops.tile([P, QR], F32, tag="wopsum", name="l2bm")
            nc.tensor.matmul(bmean_ps[:], r(ones_row[:]), r(mean[:]),
                             start=True, stop=True)
            bmean = l2p.tile([P, QR], F32, tag="ln2_bmean")
            nc.scalar.activation(bmean[:], bmean_ps[:], AF.Copy)
            brstd_ps = wops.tile([P, QR], F32, tag="wopsum", name="l2br")
            nc.tensor.matmul(brstd_ps[:], r(ones_row[:]), r(rstd[:]),
                             start=True, stop=True)
            brstd = l2p.tile([P, QR], F32, tag="ln2_brstd")
            nc.scalar.activation(brstd[:], brstd_ps[:], AF.Copy)
            for i in range(HT):
                t1 = l2p.tile([P, QR], F32, tag="ln2_t1", name="ln2_t1")
                nc.vector.tensor_sub(t1[:], h_res[:, i, :], bmean[:])
                nc.vector.tensor_mul(g_res[:, i, :], t1[:], brstd[:])

        with tc.tile_pool(name="ffn1", bufs=3) as f1p, \
             tc.tile_pool(name="f1ps", bufs=2, space="PSUM") as f1ps:
            for fM in range(FT):
                w1t = f1p.tile([P, HT, P], BF16, tag="w1t")
                nc.sync.dma_start(w1t[:], w1_t[fM])
                ps = f1ps.tile([P, QR], F32, tag="f1psum")
                for ht in range(HT):
                    nc.tensor.matmul(ps[:], w1t[:, ht, :], g_res[:, ht, :],
                                     start=(ht == 0), stop=(ht == HT - 1))
                nc.scalar.activation(f_res[:, fM, :], ps[:], AF.Gelu,
                                     bias=b1_t[:, fM:fM + 1])
        cm_g.__exit__(None, None, None)

        with tc.tile_pool(name="ffn2", bufs=3) as f2p, \
             tc.tile_pool(name="f2ps", bufs=2, space="PSUM") as f2ps:
            for dM in range(HT):
                ps = f2ps.tile([P, QR], F32, tag="f2psum")
                for q4 in range(4):
                    w2t = f2p.tile([P, HT, P], BF16, tag="w2t")
                    nc.sync.dma_start(
                        w2t[:], w2_t[dM][:, 16 * q4:16 * (q4 + 1), :])
                    for ft in range(HT):
                        kk = 16 * q4 + ft
                        nc.tensor.matmul(ps[:], w2t[:, ft, :],
                                         f_res[:, kk, :],
                                         start=(kk == 0), stop=(kk == FT - 1))
                ost = f2p.tile([P, QR], F32, tag="ost")
                nc.vector.scalar_tensor_tensor(
                    ost[:], ps[:], b2_t[:, dM:dM + 1], h_res[:, dM, :],
                    op0=OP.add, op1=OP.add)
                nc.sync.dma_start(outT[P * dM:P * (dM + 1), :], ost[:])
        cm_f.__exit__(None, None, None)
        cm_per.__exit__(None, None, None)
        cm_dram.__exit__(None, None, None)
        cm_const.__exit__(None, None, None)

    return nc


# ---------------------------------------------------------------------------
# Host side
# ---------------------------------------------------------------------------

_CACHE = {}


def _get_nc():
    if "nc" not in _CACHE:
        _CACHE["nc"] = build_nc()
    return _CACHE["nc"]


def make_in_maps(inputs):
    import ml_dtypes
    BF = ml_dtypes.bfloat16
    x = np.asarray(inputs["x"], np.float32)
    scale = np.float32(1.0 / np.sqrt(HD))
    wqkv = np.asarray(inputs["wqkv_w"], np.float32)
    wqkv_b = np.asarray(inputs["wqkv_b"], np.float32)
    ln1w = np.asarray(inputs["ln1_w"], np.float32)
    ln1b = np.asarray(inputs["ln1_b"], np.float32)
    ln2w = np.asarray(inputs["ln2_w"], np.float32)
    ln2b = np.asarray(inputs["ln2_b"], np.float32)
    w1 = np.asarray(inputs["w1"], np.float32)
    wq, wk, wv = wqkv[:H], wqkv[H:2 * H], wqkv[2 * H:]

    def tile_kxm(wT, dt=np.float32):
        # [K, M] -> [mM, p, kt, m2] so each [128, kt*128] lhsT load is
        # contiguous per partition
        K_, M_ = wT.shape
        return np.ascontiguousarray(
            wT.reshape(K_ // P, P, M_ // P, P).transpose(2, 1, 0, 3)).astype(dt)

    # LN affine params are folded into the downstream projections:
    # W @ (y*w + b) = (W*diag(w)) @ y + W @ b   (exact algebra)
    shared = {
        "ones_d": np.ones((P, 1), np.float32),
        "ones_bf_d": np.ones((P, 1), BF),
        "ones_r_d": np.ones((1, P), np.float32),
        "ident_d": np.eye(P, dtype=np.float32).astype(BF),
        "wq_t": tile_kxm(wq.T * ln1w[:, None] * scale, BF),
        "wk_t": tile_kxm(wk.T * ln1w[:, None], BF),
        "wvT": np.ascontiguousarray(wv.T * ln1w[:, None]).astype(BF),
        "wo_t": tile_kxm(np.asarray(inputs["wo_w"], np.float32).T, BF),
        "w1_t": tile_kxm(w1.T * ln2w[:, None], BF),
        "w2_t": tile_kxm(np.asarray(inputs["w2"], np.float32).T, BF),
        "bq": np.ascontiguousarray((wqkv_b[:H] + wq @ ln1b) * scale),
        "bk": np.ascontiguousarray(wqkv_b[H:2 * H] + wk @ ln1b),
        "bv": np.ascontiguousarray(wqkv_b[2 * H:] + wv @ ln1b),
        "bwo": np.asarray(inputs["wo_b"], np.float32),
        "b1": np.asarray(inputs["b1"], np.float32) + w1 @ ln2b,
        "b2": np.asarray(inputs["b2"], np.float32),
    }
    kidx = np.arange(S)
    in_maps = []
    for core in range(8):
        b, c = divmod(core, 4)
        q0 = QR * c
        qidx = q0 + np.arange(QR)
        m = np.where(kidx[:, None] <= qidx[None, :], np.float32(0),
                     np.float32(NEG)).astype(BF)
        in_maps.append(dict(
            shared,
            xTq=np.ascontiguousarray(x[b, q0:q0 + QR].T),
            maskT=np.ascontiguousarray(m),
        ))
    return in_maps


def run_cores(inputs, **run_kw):
    nc = _get_nc()
    in_maps = make_in_maps(inputs)
    return nc, run_bass_kernel_spmd(nc, in_maps, core_ids=list(range(8)),
                                    **run_kw)


def kernel(**inputs):
    _, res = run_cores(inputs)
    out = np.empty((B, S, H), np.float32)
    for core in range(8):
        b, c = divmod(core, 4)
        out[b, QR * c:QR * (c + 1), :] = res.results[core]["outT"].T
    return out
